# revision 1
# baseline (speedup 1.0000x reference)
"""GQA attention with RoPE on 8 TRN2 NeuronCores (Bass/Tile, bf16).

Sharding: head + batch tensor parallel, collectives only at the end.
  - Core c = (b=c//4, g=c%4) owns batch b, query heads 8g..8g+7 and kv
    heads {2g, 2g+1}. Projections, RoPE and causal attention for those
    heads run with ZERO cross-core communication, so the initial
    collective rendezvous/launch skew overlaps ~170us of local compute
    (the v1 kernel lost ~115us of PE idle waiting for early AllToAlls).
  - e-tiles pair heads (8g+e, 8g+4+e) so one 128-row tile holds a
    (kv 2g, kv 2g+1) head pair; scores run as two 64x128 row-tiled
    matmuls (K.T stationary, feature-major Q moving) producing S.T so
    softmax'd probs feed AV without transposes; denominators come free
    from a ones column appended to V.
  - After attention for e-tile e, one small AllToAll per e-tile within
    the 4-core batch group flips head-sharded outputs to token-sharded
    [2048 features, 512 tokens]; the output projection (full wo,
    prefetched into the freed x buffer) emits y.T for the core's 512
    tokens. All weight/activation SBUF images are prepared host-side so
    every DMA is a single contiguous 2D copy.
"""
import os
import numpy as np
import ml_dtypes

N_CORES = 8
B, L, D = 2, 2048, 2048
N_HEADS, KV_HEADS, HEAD_DIM = 32, 8, 64
THETA = 10000.0
DCH = D // 128            # 16 contraction chunks
NKB = L // 128            # 16 key blocks
CH = 256                  # q chunk
NCH = L // CH             # 8 q chunks
TSH = L // 4              # 512 output tokens per core

_BUILT = {}


def _build_nc():
    import concourse.bacc as bacc
    import concourse.tile as tile
    from concourse import mybir
    from concourse.masks import make_identity

    f32 = mybir.dt.float32
    bf16 = mybir.dt.bfloat16

    nc = bacc.Bacc("TRN2", target_bir_lowering=False, debug=False,
                   num_devices=N_CORES)

    xT_ext = nc.dram_tensor("xT", [128, DCH * L], bf16, kind="ExternalInput")
    wqT_ext = nc.dram_tensor("wqT", [128, 4 * D], bf16, kind="ExternalInput")
    wkvT_ext = nc.dram_tensor("wkvT", [128, 2 * D], bf16, kind="ExternalInput")
    woT_ext = nc.dram_tensor("woT", [128, 16 * D], bf16, kind="ExternalInput")
    cos_ext = nc.dram_tensor("cosT", [128, L], bf16, kind="ExternalInput")
    sin_ext = nc.dram_tensor("sinT", [128, L], bf16, kind="ExternalInput")
    mask_ext = nc.dram_tensor("maskT", [128, 2 * CH], bf16, kind="ExternalInput")
    yT_ext = nc.dram_tensor("yT", [128, 16 * TSH], f32, kind="ExternalOutput")

    rg = [list(range(N_CORES))]

    with tile.TileContext(nc) as tc:
        with tc.tile_pool(name="dram", bufs=1, space="DRAM") as dram, \
             tc.tile_pool(name="const", bufs=1) as const, \
             tc.tile_pool(name="kv", bufs=1) as kv, \
             tc.tile_pool(name="att", bufs=1) as att, \
             tc.tile_pool(name="ptp", bufs=3) as ptp, \
             tc.tile_pool(name="rope", bufs=4) as rp, \
             tc.tile_pool(name="nrm", bufs=4) as nrm, \
             tc.tile_pool(name="ps_proj", bufs=2, space="PSUM") as ps_proj, \
             tc.tile_pool(name="ps_sc", bufs=2, space="PSUM") as ps_sc, \
             tc.tile_pool(name="ps_av", bufs=2, space="PSUM") as ps_av:

            # One 8-core AllToAll at the end: shard j = my features (all 4
            # e-tiles) for tokens [256j, 256j+256) of MY batch; core j
            # o-projects chunk j of BOTH batches, so every (src, dst) pair
            # carries real data. A single late collective keeps the in-order
            # gpsimd queue (whose sem relays gate attention matmuls) from
            # stalling behind slow mid-kernel rendezvous.
            ain01 = dram.tile([1024, 2 * CH], bf16, tag="ain01")
            ain2 = dram.tile([1024, CH], bf16, tag="ain2")
            ain3 = dram.tile([1024, CH], bf16, tag="ain3")
            aout01 = dram.tile([1024, 2 * CH], bf16, tag="aout01")
            aout2 = dram.tile([1024, CH], bf16, tag="aout2")
            aout3 = dram.tile([1024, CH], bf16, tag="aout3")

            cos_sb = const.tile([128, L], bf16, tag="cos")
            sin_sb = const.tile([128, L], bf16, tag="sin")
            mask_sb = const.tile([128, 2 * CH], bf16, tag="mask")
            ident = const.tile([128, 128], bf16, tag="ident")
            # exp via int arithmetic (Schraudolph): bitcast(int32(s*A + B))
            expB = const.tile([128, 1024], f32, tag="expB")
            nc.any.memset(expB[:], float((127 << 23) - 486411))
            nc.gpsimd.dma_start(out=cos_sb[:], in_=cos_ext[:])
            nc.gpsimd.dma_start(out=sin_sb[:], in_=sin_ext[:])
            nc.gpsimd.dma_start(out=mask_sb[:], in_=mask_ext[:])
            make_identity(nc, ident[:])

            kT = kv.tile([128, L], bf16, tag="kT")
            vT = kv.tile([128, L], bf16, tag="vT")
            vones = [kv.tile([128, NKB * 65], bf16, name=f"vo{h}", tag=f"vo{h}")
                     for h in range(2)]
            qT = [kv.tile([128, L], bf16, name=f"qT{e}", tag=f"qT{e}")
                  for e in range(4)]
            for h in range(2):
                nc.any.memset(vones[h][:], 1.0)

            def rope_to(ps, out_slice, cslice):
                """RoPE a [128, 512] feature-major psum chunk into bf16 sbuf.
                Rows r: dim d = r%64; rotate-half via partition-shifted psum
                reads; sinT has the sign baked in host-side."""
                t1 = rp.tile([128, 512], bf16, tag="t1")
                nc.vector.tensor_mul(t1[:], ps[:], cos_sb[:, cslice])
                t2 = rp.tile([128, 512], bf16, tag="t2")
                for hh in range(2):
                    b0 = 64 * hh
                    nc.vector.tensor_mul(t2[b0:b0 + 32, :],
                                         ps[b0 + 32:b0 + 64, :],
                                         sin_sb[b0:b0 + 32, cslice])
                    nc.vector.tensor_mul(t2[b0 + 32:b0 + 64, :],
                                         ps[b0:b0 + 32, :],
                                         sin_sb[b0 + 32:b0 + 64, cslice])
                nc.vector.tensor_add(out_slice, t1[:], t2[:])

            # -------- phase 1: loads + K/V/Q projections (local) --------
            with tc.tile_pool(name="xw", bufs=1) as xw:
                xT_sb = xw.tile([128, DCH * L], bf16, tag="xT")
                wq_sb = xw.tile([128, 4 * D], bf16, tag="wq")
                wkv_sb = xw.tile([128, 2 * D], bf16, tag="wkv")
                # wkv first (KV proj starts earliest), xT split across both
                # queues, wq (e-major host layout) on gpsimd
                nc.sync.dma_start(out=wkv_sb[:], in_=wkvT_ext[:])
                for i in range(DCH):
                    q = nc.sync if i % 2 == 0 else nc.gpsimd
                    q.dma_start(out=xT_sb[:, L * i:L * (i + 1)],
                                in_=xT_ext[:, L * i:L * (i + 1)])
                nc.gpsimd.dma_start(out=wq_sb[:], in_=wqT_ext[:])

                def kv_proj(t, dst_rope):
                    for tc4 in range(4):
                        # borrow idle score-psum slots so 4 projection groups
                        # accumulate concurrently while xT chunks stream in
                        pool = ps_proj if tc4 % 2 == 0 else ps_sc
                        ps = pool.tile([128, 512], f32,
                                       tag="proj" if tc4 % 2 == 0 else "sc")
                        for i in range(DCH):
                            nc.tensor.matmul(
                                ps[:],
                                wkv_sb[:, 2048 * t + 128 * i:2048 * t + 128 * (i + 1)],
                                xT_sb[:, L * i + 512 * tc4:L * i + 512 * (tc4 + 1)],
                                start=(i == 0), stop=(i == DCH - 1))
                        sl = slice(512 * tc4, 512 * (tc4 + 1))
                        if dst_rope:
                            rope_to(ps[:], kT[:, sl], sl)
                        else:
                            nc.scalar.copy(vT[:, sl], ps[:])

                def q_proj(e, tc4):
                    ps = ps_proj.tile([128, 512], f32, tag="proj")
                    for i in range(DCH):
                        nc.tensor.matmul(
                            ps[:],
                            wq_sb[:, 2048 * e + 128 * i:2048 * e + 128 * (i + 1)],
                            xT_sb[:, L * i + 512 * tc4:L * i + 512 * (tc4 + 1)],
                            start=(i == 0), stop=(i == DCH - 1))
                    sl = slice(512 * tc4, 512 * (tc4 + 1))
                    rope_to(ps[:], qT[e][:, sl], sl)

                kv_proj(0, True)   # K
                kv_proj(1, False)  # V
                # V transposes: [128 dims, 128 keys] -> [128 keys, 128 dims]
                for kb in range(NKB):
                    tp = ps_proj.tile([128, 128], bf16, tag="proj")
                    nc.tensor.transpose(tp[:], vT[:, 128 * kb:128 * (kb + 1)],
                                        ident[:])
                    nc.vector.tensor_copy(vones[0][:, 65 * kb:65 * kb + 64],
                                          tp[:, 0:64])
                    nc.vector.tensor_copy(vones[1][:, 65 * kb:65 * kb + 64],
                                          tp[:, 64:128])
                for tc4 in range(4):
                    q_proj(0, tc4)

                # -------- phase 2: attention per e-tile --------
                def ptcol(c, kb, hh):
                    """pt column of key block kb, head-half hh for chunk c.
                    Full groups of 4 blocks hold [hh0 x4 | hh1 x4]; a 2-block
                    tail packs [hh0 x2 | hh1 x2] so ONE exp covers both."""
                    nkb = 2 * (c + 1)
                    k4, s = kb // 4, kb % 4
                    if 4 * k4 + 4 <= nkb:
                        return 2048 * k4 + 1024 * hh + 256 * s
                    return 2048 * k4 + 512 * hh + 256 * s

                EXPA = float((1 << 23) / np.log(2.0) * 0.125)

                def emit_exp(dst, ssp, on_dve):
                    if not on_dve:
                        nc.scalar.activation(
                            dst, ssp, mybir.ActivationFunctionType.Exp,
                            scale=0.125)
                        return
                    # DVE offload: exp(s/8) ~= bitcast_f32(int32(s*A + B));
                    # +-2% sawtooth, fine at rel-err budget; keeps ACT from
                    # rate-limiting the attention pipeline.
                    n = ssp.shape[1]
                    i32 = att.tile([128, 1024], mybir.dt.int32, tag="i32")
                    nc.vector.scalar_tensor_tensor(
                        i32[:, 0:n], ssp, EXPA, expB[:, 0:n],
                        mybir.AluOpType.mult, mybir.AluOpType.add)
                    nc.vector.tensor_copy(dst, i32[:, 0:n].bitcast(f32))

                def scores_chunk(e, c):
                    """S.T for q chunk c, both head-halves -> exp'd probs."""
                    nkb = 2 * (c + 1)
                    pt = ptp.tile([128, 4 * 2048], bf16, tag="pt",
                                  name=f"pt_{e}_{c}")
                    for k4 in range(nkb // 4):
                        for hh in range(2):
                            h0 = 64 * hh
                            ssp = ps_sc.tile([128, 1024], f32, tag="sc")
                            for s in range(4):
                                kb = 4 * k4 + s
                                nc.tensor.matmul(
                                    ssp[:, 256 * s:256 * (s + 1)],
                                    kT[h0:h0 + 64, 128 * kb:128 * (kb + 1)],
                                    qT[e][h0:h0 + 64, CH * c:CH * (c + 1)],
                                    start=True, stop=True)
                            emit_exp(
                                pt[:, 2048 * k4 + 1024 * hh:
                                   2048 * k4 + 1024 * (hh + 1)],
                                ssp[:], on_dve=(hh == 1 and c % 2 == 1))
                    if nkb % 4:
                        k4 = nkb // 4
                        ssp = ps_sc.tile([128, 1024], f32, tag="sc")
                        for hh in range(2):
                            h0 = 64 * hh
                            for s in range(2):
                                kb = 4 * k4 + s
                                nc.tensor.matmul(
                                    ssp[:, 256 * (2 * hh + s):
                                        256 * (2 * hh + s + 1)],
                                    kT[h0:h0 + 64, 128 * kb:128 * (kb + 1)],
                                    qT[e][h0:h0 + 64, CH * c:CH * (c + 1)],
                                    start=True, stop=True)
                        nc.scalar.activation(
                            pt[:, 2048 * k4:2048 * k4 + 1024], ssp[:],
                            mybir.ActivationFunctionType.Exp, scale=0.125)
                    for hh in range(2):
                        c0 = ptcol(c, 2 * c, hh)
                        nc.vector.tensor_mul(pt[:, c0:c0 + 512],
                                             pt[:, c0:c0 + 512], mask_sb[:])
                    return pt

                def av_chunk(e, c, hh, pt, avt, an_t, _ptmem={}):
                    if pt is not None:
                        _ptmem['pt'] = pt
                    pt = _ptmem['pt']
                    nkb = 2 * (c + 1)
                    av = avt[:, 256 * hh:256 * (hh + 1)]
                    for kb in range(nkb):
                        c0 = ptcol(c, kb, hh)
                        nc.tensor.matmul(
                            av,
                            vones[hh][:, 65 * kb:65 * kb + 65],
                            pt[:, c0:c0 + 256],
                            start=(kb == 0), stop=(kb == nkb - 1))
                    lrow = nrm.tile([1, 256], f32, tag="lrow")
                    nc.vector.tensor_copy(lrow[:], avt[64:65, 256 * hh:256 * (hh + 1)])
                    linv = nrm.tile([1, 256], f32, tag="linv")
                    nc.vector.reciprocal_approx_fast(out=linv[:], in_=lrow[:])
                    bcs = nrm.tile([64, 256], f32, tag="bcs")
                    nc.gpsimd.partition_broadcast(bcs[:], linv[0:1, :],
                                                  channels=64)
                    nc.vector.tensor_mul(
                        an_t[64 * hh:64 * hh + 64, :],
                        avt[0:64, 256 * hh:256 * (hh + 1)], bcs[:])

                def att_etile(e, fill, fill_cs=(4, 5, 6, 7)):
                    """Attention for e-tile e; fill(k) emits PE filler work
                    (next Q projection / o-proj chunks) at the late chunks
                    where ACT lags."""
                    ain_t = (ain01, ain01, ain2, ain3)[e]
                    ecol = CH * e if e < 2 else 0
                    dq = nc.sync
                    pts = {}

                    def do_av(pc):
                        avt = ps_av.tile([65, 512], f32, tag="av")
                        an_t = nrm.tile([128, CH], bf16, tag="an")
                        av_chunk(e, pc, 0, pts.pop(pc), avt, an_t)
                        av_chunk(e, pc, 1, None, avt, an_t)
                        dq.dma_start(
                            out=ain_t[128 * pc:128 * (pc + 1), ecol:ecol + CH],
                            in_=an_t[:])

                    for c in range(NCH):
                        pts[c] = scores_chunk(e, c)
                        if fill is not None and c in fill_cs:
                            fill(fill_cs.index(c))
                        if c >= 2:
                            do_av(c - 2)   # 2-chunk lag hides exp+mask latency
                    do_av(NCH - 2)
                    do_av(NCH - 1)

                att_etile(0, lambda t4: q_proj(1, t4))
                att_etile(1, lambda t4: q_proj(2, t4))
                att_etile(2, lambda t4: q_proj(3, t4))
                p_after_e2 = tc.cur_priority

            # -------- phase 3: last e-tile with o-proj filler -------------
            # The e0/e1/e2 exchange completes early in att(e3); o-proj over
            # those 12 feature chunks becomes e3's PE filler + tail overlap
            # for the final (small) e3-only exchange; after it only 4 chunks
            # per dt remain.
            with tc.tile_pool(name="wop", bufs=1) as wop, \
                 tc.tile_pool(name="yo", bufs=4) as yo:
                wo_sb = wop.tile([128, 8 * D], bf16, tag="wo")
                aout_sb = wop.tile([128, 16 * TSH], bf16, tag="aout")
                yhalf = wop.tile([128, 16 * 512], bf16, tag="yhalf")
                for jj in range(8):
                    q = nc.gpsimd if jj % 2 == 0 else nc.sync
                    q.dma_start(out=wo_sb[:, D * jj:D * (jj + 1)],
                                in_=woT_ext[:, D * jj:D * (jj + 1)])
                # pin the trigger's schedule slot to just after att(e2) so
                # the scheduler cannot order it behind att(e3)'s gpsimd work
                with tc.high_priority(offset=tc.cur_priority - p_after_e2):
                    nc.gpsimd.collective_compute(
                        "AllToAll", mybir.AluOpType.bypass, replica_groups=rg,
                        ins=[ain01[:].opt()], outs=[aout01[:].opt()])
                    nc.gpsimd.collective_compute(
                        "AllToAll", mybir.AluOpType.bypass, replica_groups=rg,
                        ins=[ain2[:].opt()], outs=[aout2[:].opt()])

                # aout_sb col 512*(4e+gg) = [b0 256 | b1 256] for (e, gg)
                def aout_load(e):
                    at = (aout01, aout01, aout2, aout3)[e]
                    ecol = CH * e if e < 2 else 0
                    for j in range(8):
                        b2, gg = j // 4, j % 4
                        dst = 512 * (4 * e + gg) + 256 * b2
                        nc.sync.dma_start(
                            out=aout_sb[:, dst:dst + 256],
                            in_=at[128 * j:128 * (j + 1), ecol:ecol + CH])

                aout_load(0)
                aout_load(1)

                DT_SPLIT = ((0, 5), (5, 10), (10, 16))  # 2 in-attention fills + 1 after

                def oproj_a(k):
                    for dt in range(*DT_SPLIT[k]):
                        ps = ps_proj.tile([128, 512], f32, tag="proj")
                        for jj in range(8):
                            nc.tensor.matmul(
                                ps[:],
                                wo_sb[:, D * jj + 128 * dt:D * jj + 128 * (dt + 1)],
                                aout_sb[:, 512 * jj:512 * (jj + 1)],
                                start=(jj == 0), stop=(jj == 7))
                        nc.scalar.copy(yhalf[:, 512 * dt:512 * (dt + 1)], ps[:])

                att_etile(3, oproj_a, fill_cs=(6, 7))
                p_after_e3 = tc.cur_priority
                oproj_a(2)
                # e2's features (landed during e3) accumulate into yhalf now
                wo_sb2 = wop.tile([128, 4 * D], bf16, tag="wo")
                for jj in range(4):
                    q = nc.gpsimd if jj % 2 == 0 else nc.sync
                    q.dma_start(out=wo_sb2[:, D * jj:D * (jj + 1)],
                                in_=woT_ext[:, D * (jj + 8):D * (jj + 9)])
                aout_load(2)
                for dt in range(DCH):
                    ps = ps_proj.tile([128, 512], f32, tag="proj")
                    for nn in range(4):
                        jj = nn + 8
                        nc.tensor.matmul(
                            ps[:],
                            wo_sb2[:, D * nn + 128 * dt:D * nn + 128 * (dt + 1)],
                            aout_sb[:, 512 * jj:512 * (jj + 1)],
                            start=(nn == 0), stop=(nn == 3))
                    nc.vector.tensor_add(yhalf[:, 512 * dt:512 * (dt + 1)],
                                         ps[:], yhalf[:, 512 * dt:512 * (dt + 1)])
                with tc.high_priority(offset=tc.cur_priority - p_after_e3):
                    nc.gpsimd.collective_compute(
                        "AllToAll", mybir.AluOpType.bypass, replica_groups=rg,
                        ins=[ain3[:].opt()], outs=[aout3[:].opt()])
                wo_sb3 = wop.tile([128, 4 * D], bf16, tag="wo")
                for jj in range(4):
                    q = nc.gpsimd if jj % 2 == 0 else nc.sync
                    q.dma_start(out=wo_sb3[:, D * jj:D * (jj + 1)],
                                in_=woT_ext[:, D * (jj + 12):D * (jj + 13)])
                aout_load(3)
                for dt in range(DCH):
                    ps = ps_proj.tile([128, 512], f32, tag="proj")
                    for nn in range(4):
                        jj = nn + 12
                        nc.tensor.matmul(
                            ps[:],
                            wo_sb3[:, D * nn + 128 * dt:D * nn + 128 * (dt + 1)],
                            aout_sb[:, 512 * jj:512 * (jj + 1)],
                            start=(nn == 0), stop=(nn == 3))
                    yv = yo.tile([128, 512], f32, tag="y")
                    nc.vector.tensor_add(yv[:], ps[:],
                                         yhalf[:, 512 * dt:512 * (dt + 1)])
                    nc.sync.dma_start(
                        out=yT_ext[:, 512 * dt:512 * (dt + 1)],
                        in_=yv[:])

    nc.compile()
    return nc


def _host_inputs(x, wq, wk, wv, wo):
    bf = ml_dtypes.bfloat16

    # xT per batch: [128, 16*2048]; img[p, 2048i + t] = x[b, t, 128i + p]
    xT = []
    for b in range(B):
        t = x[b].T.reshape(DCH, 128, L).transpose(1, 0, 2).reshape(128, DCH * L)
        xT.append(np.ascontiguousarray(t).astype(bf))

    # wq per group g: e-major image; col 2048e + 128i + r ; row = qrow(g,e,r)
    wqT = []
    for g in range(4):
        img = np.empty((128, 4 * D), np.float32)
        for e in range(4):
            rows = np.concatenate([
                np.arange(64) + 64 * (8 * g + e),
                np.arange(64) + 64 * (8 * g + 4 + e)])
            Wsel = wq[rows, :]                      # [128, 2048]
            blk = Wsel.T.reshape(DCH, 128, 128)     # [i, p, r]
            img[:, 2048 * e:2048 * (e + 1)] = \
                blk.transpose(1, 0, 2).reshape(128, 2048)
        wqT.append(np.ascontiguousarray(img).astype(bf))

    # wkv per group g: t-major (K then V); rows 128g..128g+128 of wk/wv
    wkvT = []
    for g in range(4):
        img = np.empty((128, 2 * D), np.float32)
        for t, W in enumerate((wk, wv)):
            Wsel = W[128 * g:128 * (g + 1), :]
            blk = Wsel.T.reshape(DCH, 128, 128)
            img[:, 2048 * t:2048 * (t + 1)] = \
                blk.transpose(1, 0, 2).reshape(128, 2048)
        wkvT.append(np.ascontiguousarray(img).astype(bf))

    # wo (shared): col 2048jj + eo, row p; f(jj=4e+r, p)
    forder = np.empty(2048, np.int64)
    for jj in range(16):
        e, r = jj // 4, jj % 4
        p = np.arange(128)
        head = np.where(p < 64, 8 * r + e, 8 * r + 4 + e)
        forder[128 * jj:128 * (jj + 1)] = 64 * head + (p % 64)
    Wsel = wo[:, forder]                            # [2048 eo, 2048 f]
    woT = np.ascontiguousarray(
        Wsel.T.reshape(16, 128, D).transpose(1, 0, 2).reshape(128, 16 * D)
    ).astype(bf)

    # rope tables [128, 2048]: row r -> dim d = r%64
    freqs = 1.0 / (THETA ** (np.arange(0, HEAD_DIM, 2, dtype=np.float32) / HEAD_DIM))
    pos = np.arange(L, dtype=np.float32)
    ph = np.outer(freqs, pos)                       # [32, L]
    cos64 = np.concatenate([np.cos(ph), np.cos(ph)], axis=0)   # [64, L]
    sin64 = np.concatenate([np.sin(ph), np.sin(ph)], axis=0)
    sgn = np.where(np.arange(64) < 32, -1.0, 1.0)[:, None].astype(np.float32)
    cosT = np.concatenate([cos64, cos64], axis=0).astype(bf)
    sinT = np.concatenate([sin64 * sgn, sin64 * sgn], axis=0).astype(bf)

    q_idx = np.arange(CH)
    k_idx = np.arange(128)
    m0 = np.where(k_idx[:, None] <= q_idx[None, :], 1.0, 0.0)
    m1 = np.where(k_idx[:, None] + 128 <= q_idx[None, :], 1.0, 0.0)
    mask = np.concatenate([m0, m1], axis=1).astype(bf)

    in_maps = []
    for c in range(N_CORES):
        b, g = c // 4, c % 4
        in_maps.append({
            "xT": xT[b], "wqT": wqT[g], "wkvT": wkvT[g], "woT": woT,
            "cosT": cosT, "sinT": sinT, "maskT": mask,
        })
    return in_maps


def kernel(x, wq, wk, wv, wo):
    from concourse.bass_utils import run_bass_kernel_spmd

    if "nc" not in _BUILT:
        _BUILT["nc"] = _build_nc()
    nc = _BUILT["nc"]

    in_maps = _host_inputs(np.asarray(x), np.asarray(wq), np.asarray(wk),
                           np.asarray(wv), np.asarray(wo))
    trace = bool(os.environ.get("BASS_KERNEL_TRACE"))
    res = run_bass_kernel_spmd(nc, in_maps, core_ids=list(range(N_CORES)),
                               trace=trace)
    kernel.last_exec_time_ns = res.exec_time_ns
    kernel.last_results = res

    y = np.empty((B, L, D), dtype=np.float32)
    for c in range(N_CORES):
        arr = res.results[c]["yT"].reshape(128, DCH, 2, CH)
        for b2 in range(2):
            y[b2, CH * c:CH * (c + 1), :] = \
                arr[:, :, b2, :].transpose(2, 1, 0).reshape(CH, D)
    return y



# revision 4
# speedup vs baseline: 1.0454x; 1.0454x over previous
"""GQA attention with RoPE on 8 TRN2 NeuronCores (Bass/Tile, bf16).

Sharding: head + batch tensor parallel.
  - Core c = (b=c//4, g=c%4) owns batch b, query heads 8g..8g+7 and kv
    heads {2g, 2g+1}. Projections, RoPE and causal attention for those
    heads run with ZERO cross-core communication.
  - e-tiles pair heads (8g+e, 8g+4+e) so one 128-row tile holds a
    (kv 2g, kv 2g+1) head pair; scores run as two 64x128 row-tiled
    matmuls (K.T stationary, feature-major Q moving) producing S.T so
    softmax'd probs feed AV without transposes; denominators come free
    from a ones column appended to V.
  - A tiny warm-up AllToAll at kernel start absorbs cross-core launch
    skew on the collective queue, so real AllToAlls rendezvous fast.
    Each e-tile's AllToAll triggers as soon as its data is complete
    (e0+e1 share one buffer -> after e1; e2 after e2; e3 after e3),
    overlapping the exchange with the next e-tile's attention.
  - Fillers: e0 carries q_proj(1)+q_proj(2), e1 carries q_proj(3) (so
    the x/wq SBUF pool frees after e1 and wo/aout load during e2);
    o-proj feature chunks fill e2 (dt 0-5) and e3 (dt 6-15); the tail
    is pass2 (e2's features, overlapping the last AllToAll) + pass3.
"""
import os
import numpy as np
import ml_dtypes

N_CORES = 8
B, L, D = 2, 2048, 2048
N_HEADS, KV_HEADS, HEAD_DIM = 32, 8, 64
THETA = 10000.0
DCH = D // 128            # 16 contraction chunks
NKB = L // 128            # 16 key blocks
CH = 256                  # q chunk
NCH = L // CH             # 8 q chunks
TSH = L // 4              # 512 output tokens per core

_BUILT = {}


def _build_nc():
    import concourse.bacc as bacc
    import concourse.tile as tile
    from concourse import mybir
    from concourse.masks import make_identity

    f32 = mybir.dt.float32
    bf16 = mybir.dt.bfloat16

    nc = bacc.Bacc("TRN2", target_bir_lowering=False, debug=False,
                   num_devices=N_CORES)

    xT_ext = nc.dram_tensor("xT", [128, DCH * L], bf16, kind="ExternalInput")
    wqT_ext = nc.dram_tensor("wqT", [128, 4 * D], bf16, kind="ExternalInput")
    wkvT_ext = nc.dram_tensor("wkvT", [128, 2 * D], bf16, kind="ExternalInput")
    woT_ext = nc.dram_tensor("woT", [128, 16 * D], bf16, kind="ExternalInput")
    cos_ext = nc.dram_tensor("cosT", [128, L], bf16, kind="ExternalInput")
    sin_ext = nc.dram_tensor("sinT", [128, L], bf16, kind="ExternalInput")
    mask_ext = nc.dram_tensor("maskT", [128, 2 * CH], bf16, kind="ExternalInput")
    yT_ext = nc.dram_tensor("yT", [128, 16 * TSH], f32, kind="ExternalOutput")

    rg = [list(range(N_CORES))]

    with tile.TileContext(nc) as tc:
        with tc.tile_pool(name="dram", bufs=1, space="DRAM") as dram, \
             tc.tile_pool(name="const", bufs=1) as const, \
             tc.tile_pool(name="kv", bufs=1) as kv, \
             tc.tile_pool(name="att", bufs=1) as att, \
             tc.tile_pool(name="ptp", bufs=3) as ptp, \
             tc.tile_pool(name="rope", bufs=4) as rp, \
             tc.tile_pool(name="nrm", bufs=4) as nrm, \
             tc.tile_pool(name="ps_proj", bufs=2, space="PSUM") as ps_proj, \
             tc.tile_pool(name="ps_sc", bufs=2, space="PSUM") as ps_sc, \
             tc.tile_pool(name="ps_av", bufs=2, space="PSUM") as ps_av:

            warm_i = dram.tile([8, 16], bf16, tag="warm_i")
            warm_o = dram.tile([8, 16], bf16, tag="warm_o")
            ain01 = dram.tile([1024, 2 * CH], bf16, tag="ain01")
            ain2 = dram.tile([1024, CH], bf16, tag="ain2")
            ain3 = dram.tile([1024, CH], bf16, tag="ain3")
            aout01 = dram.tile([1024, 2 * CH], bf16, tag="aout01")
            aout2 = dram.tile([1024, CH], bf16, tag="aout2")
            aout3 = dram.tile([1024, CH], bf16, tag="aout3")

            # Warm-up collective: rendezvous-only. All 8 cores meet here
            # at kernel start, so the launch skew is paid on the CC queue
            # while the compute engines stream phase 1, instead of inside
            # the first real AllToAll mid-kernel.
            nc.gpsimd.collective_compute(
                "AllToAll", mybir.AluOpType.bypass, replica_groups=rg,
                ins=[warm_i[:].opt()], outs=[warm_o[:].opt()])

            cos_sb = const.tile([128, L], bf16, tag="cos")
            sin_sb = const.tile([128, L], bf16, tag="sin")
            mask_sb = const.tile([128, 2 * CH], bf16, tag="mask")
            ident = const.tile([128, 128], bf16, tag="ident")
            # exp via int arithmetic (Schraudolph): bitcast(int32(s*A + B))
            expB = const.tile([128, 1024], f32, tag="expB")
            nc.any.memset(expB[:], float((127 << 23) - 486411))
            nc.gpsimd.dma_start(out=cos_sb[:], in_=cos_ext[:])
            nc.gpsimd.dma_start(out=sin_sb[:], in_=sin_ext[:])
            nc.gpsimd.dma_start(out=mask_sb[:], in_=mask_ext[:])
            make_identity(nc, ident[:])

            kT = kv.tile([128, L], bf16, tag="kT")
            vT = kv.tile([128, L], bf16, tag="vT")
            vones = [kv.tile([128, NKB * 65], bf16, name=f"vo{h}", tag=f"vo{h}")
                     for h in range(2)]
            qT = [kv.tile([128, L], bf16, name=f"qT{e}", tag=f"qT{e}")
                  for e in range(4)]
            for h in range(2):
                nc.any.memset(vones[h][:], 1.0)

            def rope_to(ps, out_slice, cslice):
                """RoPE a [128, 512] feature-major psum chunk into bf16 sbuf.
                Rows r: dim d = r%64; rotate-half via partition-shifted psum
                reads; sinT has the sign baked in host-side."""
                t1 = rp.tile([128, 512], bf16, tag="t1")
                nc.vector.tensor_mul(t1[:], ps[:], cos_sb[:, cslice])
                t2 = rp.tile([128, 512], bf16, tag="t2")
                for hh in range(2):
                    b0 = 64 * hh
                    nc.vector.tensor_mul(t2[b0:b0 + 32, :],
                                         ps[b0 + 32:b0 + 64, :],
                                         sin_sb[b0:b0 + 32, cslice])
                    nc.vector.tensor_mul(t2[b0 + 32:b0 + 64, :],
                                         ps[b0:b0 + 32, :],
                                         sin_sb[b0 + 32:b0 + 64, cslice])
                nc.vector.tensor_add(out_slice, t1[:], t2[:])

            # shared attention helpers (defined once, used for all e-tiles)
            def ptcol(c, kb, hh):
                """pt column of key block kb, head-half hh for chunk c.
                Full groups of 4 blocks hold [hh0 x4 | hh1 x4]; a 2-block
                tail packs [hh0 x2 | hh1 x2] so ONE exp covers both."""
                nkb = 2 * (c + 1)
                k4, s = kb // 4, kb % 4
                if 4 * k4 + 4 <= nkb:
                    return 2048 * k4 + 1024 * hh + 256 * s
                return 2048 * k4 + 512 * hh + 256 * s

            EXPA = float((1 << 23) / np.log(2.0) * 0.125)

            def emit_exp(dst, ssp, on_dve):
                if not on_dve:
                    nc.scalar.activation(
                        dst, ssp, mybir.ActivationFunctionType.Exp,
                        scale=0.125)
                    return
                # DVE offload: exp(s/8) ~= bitcast_f32(int32(s*A + B));
                # +-2% sawtooth, fine at rel-err budget; keeps ACT from
                # rate-limiting the attention pipeline.
                n = ssp.shape[1]
                i32 = att.tile([128, 1024], mybir.dt.int32, tag="i32")
                nc.vector.scalar_tensor_tensor(
                    i32[:, 0:n], ssp, EXPA, expB[:, 0:n],
                    mybir.AluOpType.mult, mybir.AluOpType.add)
                nc.vector.tensor_copy(dst, i32[:, 0:n].bitcast(f32))

            def scores_chunk(e, c):
                """S.T for q chunk c, both head-halves -> exp'd probs."""
                nkb = 2 * (c + 1)
                pt = ptp.tile([128, 4 * 2048], bf16, tag="pt",
                              name=f"pt_{e}_{c}")
                for k4 in range(nkb // 4):
                    for hh in range(2):
                        h0 = 64 * hh
                        ssp = ps_sc.tile([128, 1024], f32, tag="sc")
                        for s in range(4):
                            kb = 4 * k4 + s
                            nc.tensor.matmul(
                                ssp[:, 256 * s:256 * (s + 1)],
                                kT[h0:h0 + 64, 128 * kb:128 * (kb + 1)],
                                qT[e][h0:h0 + 64, CH * c:CH * (c + 1)],
                                start=True, stop=True)
                        emit_exp(
                            pt[:, 2048 * k4 + 1024 * hh:
                               2048 * k4 + 1024 * (hh + 1)],
                            ssp[:], on_dve=(hh == 1 and c % 2 == 1))
                if nkb % 4:
                    k4 = nkb // 4
                    ssp = ps_sc.tile([128, 1024], f32, tag="sc")
                    for hh in range(2):
                        h0 = 64 * hh
                        for s in range(2):
                            kb = 4 * k4 + s
                            nc.tensor.matmul(
                                ssp[:, 256 * (2 * hh + s):
                                    256 * (2 * hh + s + 1)],
                                kT[h0:h0 + 64, 128 * kb:128 * (kb + 1)],
                                qT[e][h0:h0 + 64, CH * c:CH * (c + 1)],
                                start=True, stop=True)
                    nc.scalar.activation(
                        pt[:, 2048 * k4:2048 * k4 + 1024], ssp[:],
                        mybir.ActivationFunctionType.Exp, scale=0.125)
                for hh in range(2):
                    c0 = ptcol(c, 2 * c, hh)
                    nc.vector.tensor_mul(pt[:, c0:c0 + 512],
                                         pt[:, c0:c0 + 512], mask_sb[:])
                return pt

            def av_chunk(e, c, hh, pt, avt, an_t, _ptmem={}):
                if pt is not None:
                    _ptmem['pt'] = pt
                pt = _ptmem['pt']
                nkb = 2 * (c + 1)
                av = avt[:, 256 * hh:256 * (hh + 1)]
                for kb in range(nkb):
                    c0 = ptcol(c, kb, hh)
                    nc.tensor.matmul(
                        av,
                        vones[hh][:, 65 * kb:65 * kb + 65],
                        pt[:, c0:c0 + 256],
                        start=(kb == 0), stop=(kb == nkb - 1))
                lrow = nrm.tile([1, 256], f32, tag="lrow")
                nc.vector.tensor_copy(lrow[:], avt[64:65, 256 * hh:256 * (hh + 1)])
                linv = nrm.tile([1, 256], f32, tag="linv")
                nc.vector.reciprocal_approx_fast(out=linv[:], in_=lrow[:])
                bcs = nrm.tile([64, 256], f32, tag="bcs")
                nc.gpsimd.partition_broadcast(bcs[:], linv[0:1, :],
                                              channels=64)
                nc.vector.tensor_mul(
                    an_t[64 * hh:64 * hh + 64, :],
                    avt[0:64, 256 * hh:256 * (hh + 1)], bcs[:])

            def att_etile(e, fills_at=None):
                """Attention for e-tile e; fills_at maps chunk index ->
                list of thunks emitting PE filler work at that chunk."""
                fills_at = fills_at or {}
                ain_t = (ain01, ain01, ain2, ain3)[e]
                ecol = CH * e if e < 2 else 0
                dq = nc.gpsimd
                pts = {}

                def do_av(pc):
                    avt = ps_av.tile([65, 512], f32, tag="av")
                    an_t = nrm.tile([128, CH], bf16, tag="an")
                    av_chunk(e, pc, 0, pts.pop(pc), avt, an_t)
                    av_chunk(e, pc, 1, None, avt, an_t)
                    dq.dma_start(
                        out=ain_t[128 * pc:128 * (pc + 1), ecol:ecol + CH],
                        in_=an_t[:])

                for c in range(NCH):
                    pts[c] = scores_chunk(e, c)
                    for th in fills_at.get(c, ()):
                        th()
                    if c >= 2:
                        do_av(c - 2)   # 2-chunk lag hides exp+mask latency
                do_av(NCH - 2)
                do_av(NCH - 1)

            # -------- phase 1: loads + K/V/Q projections (local) --------
            with tc.tile_pool(name="xw", bufs=1) as xw:
                xT_sb = xw.tile([128, DCH * L], bf16, tag="xT")
                wq_sb = xw.tile([128, 4 * D], bf16, tag="wq")
                wkv_sb = xw.tile([128, 2 * D], bf16, tag="wkv")
                # wkv first (KV proj starts earliest), xT split across both
                # queues, wq (e-major host layout) on gpsimd
                nc.sync.dma_start(out=wkv_sb[:], in_=wkvT_ext[:])
                for i in range(DCH):
                    q = nc.sync if i % 2 == 0 else nc.gpsimd
                    q.dma_start(out=xT_sb[:, L * i:L * (i + 1)],
                                in_=xT_ext[:, L * i:L * (i + 1)])
                nc.gpsimd.dma_start(out=wq_sb[:], in_=wqT_ext[:])

                def kv_proj(t, dst_rope):
                    for tc4 in range(4):
                        # borrow idle score-psum slots so 4 projection groups
                        # accumulate concurrently while xT chunks stream in
                        pool = ps_proj if tc4 % 2 == 0 else ps_sc
                        ps = pool.tile([128, 512], f32,
                                       tag="proj" if tc4 % 2 == 0 else "sc")
                        for i in range(DCH):
                            nc.tensor.matmul(
                                ps[:],
                                wkv_sb[:, 2048 * t + 128 * i:2048 * t + 128 * (i + 1)],
                                xT_sb[:, L * i + 512 * tc4:L * i + 512 * (tc4 + 1)],
                                start=(i == 0), stop=(i == DCH - 1))
                        sl = slice(512 * tc4, 512 * (tc4 + 1))
                        if dst_rope:
                            rope_to(ps[:], kT[:, sl], sl)
                        else:
                            nc.scalar.copy(vT[:, sl], ps[:])

                def q_proj(e, tc4):
                    ps = ps_proj.tile([128, 512], f32, tag="proj")
                    for i in range(DCH):
                        nc.tensor.matmul(
                            ps[:],
                            wq_sb[:, 2048 * e + 128 * i:2048 * e + 128 * (i + 1)],
                            xT_sb[:, L * i + 512 * tc4:L * i + 512 * (tc4 + 1)],
                            start=(i == 0), stop=(i == DCH - 1))
                    sl = slice(512 * tc4, 512 * (tc4 + 1))
                    rope_to(ps[:], qT[e][:, sl], sl)

                kv_proj(0, True)   # K
                kv_proj(1, False)  # V
                # V transposes: [128 dims, 128 keys] -> [128 keys, 128 dims]
                for kb in range(NKB):
                    tp = ps_proj.tile([128, 128], bf16, tag="proj")
                    nc.tensor.transpose(tp[:], vT[:, 128 * kb:128 * (kb + 1)],
                                        ident[:])
                    nc.vector.tensor_copy(vones[0][:, 65 * kb:65 * kb + 64],
                                          tp[:, 0:64])
                    nc.vector.tensor_copy(vones[1][:, 65 * kb:65 * kb + 64],
                                          tp[:, 64:128])
                for tc4 in range(4):
                    q_proj(0, tc4)

                # -------- phase 2: attention e0/e1 (q fills drain xw) ----
                att_etile(0, {2: [lambda: q_proj(1, 0), lambda: q_proj(1, 1)],
                              3: [lambda: q_proj(1, 2), lambda: q_proj(1, 3)],
                              4: [lambda: q_proj(2, 0), lambda: q_proj(2, 1)],
                              5: [lambda: q_proj(2, 2), lambda: q_proj(2, 3)]})
                att_etile(1, {4: [lambda: q_proj(3, 0)],
                              5: [lambda: q_proj(3, 1)],
                              6: [lambda: q_proj(3, 2)],
                              7: [lambda: q_proj(3, 3)]})
                # e0+e1 head-sharded outputs are complete: exchange now,
                # overlapping att(e2). With the warm-up A2A having absorbed
                # launch skew, rendezvous here is short.
                nc.gpsimd.collective_compute(
                    "AllToAll", mybir.AluOpType.bypass, replica_groups=rg,
                    ins=[ain01[:].opt()], outs=[aout01[:].opt()])

            # -------- phase 3: wo/aout staging + e2/e3 with o-proj fill --
            with tc.tile_pool(name="wop", bufs=1) as wop, \
                 tc.tile_pool(name="yo", bufs=4) as yo:
                wo_sb = wop.tile([128, 8 * D], bf16, tag="wo")
                aout_sb = wop.tile([128, 16 * TSH], bf16, tag="aout")
                yhalf = wop.tile([128, 16 * 512], bf16, tag="yhalf")
                for jj in range(8):
                    nc.sync.dma_start(out=wo_sb[:, D * jj:D * (jj + 1)],
                                      in_=woT_ext[:, D * jj:D * (jj + 1)])

                # aout_sb col 512*(4e+gg) = [b0 256 | b1 256] for (e, gg)
                def aout_load(e):
                    at = (aout01, aout01, aout2, aout3)[e]
                    ecol = CH * e if e < 2 else 0
                    for j in range(8):
                        b2, gg = j // 4, j % 4
                        dst = 512 * (4 * e + gg) + 256 * b2
                        nc.sync.dma_start(
                            out=aout_sb[:, dst:dst + 256],
                            in_=at[128 * j:128 * (j + 1), ecol:ecol + CH])

                aout_load(0)
                aout_load(1)

                def oproj_dt(dt):
                    """y.T chunk dt from features jj 0-7 (e0+e1) -> yhalf."""
                    ps = ps_proj.tile([128, 512], f32, tag="proj")
                    for jj in range(8):
                        nc.tensor.matmul(
                            ps[:],
                            wo_sb[:, D * jj + 128 * dt:D * jj + 128 * (dt + 1)],
                            aout_sb[:, 512 * jj:512 * (jj + 1)],
                            start=(jj == 0), stop=(jj == 7))
                    nc.scalar.copy(yhalf[:, 512 * dt:512 * (dt + 1)], ps[:])

                def op2(d0):
                    return lambda: (oproj_dt(d0), oproj_dt(d0 + 1))

                att_etile(2, {5: [op2(0)], 6: [op2(2)], 7: [op2(4)]})
                nc.gpsimd.collective_compute(
                    "AllToAll", mybir.AluOpType.bypass, replica_groups=rg,
                    ins=[ain2[:].opt()], outs=[aout2[:].opt()])
                aout_load(2)

                att_etile(3, {1: [op2(6)], 2: [op2(8)], 3: [op2(10)],
                              4: [op2(12)], 5: [op2(14)]})
                nc.gpsimd.collective_compute(
                    "AllToAll", mybir.AluOpType.bypass, replica_groups=rg,
                    ins=[ain3[:].opt()], outs=[aout3[:].opt()])

                # wo jj 8-15 reuses wo_sb's slot once the e2/e3 o-proj
                # fills have drained it (late in e3)
                wo_b = wop.tile([128, 8 * D], bf16, tag="wo")
                for jj in range(8):
                    nc.sync.dma_start(out=wo_b[:, D * jj:D * (jj + 1)],
                                      in_=woT_ext[:, D * (jj + 8):D * (jj + 9)])

                # pass2: e2's features accumulate while the e3 AllToAll runs
                for dt in range(DCH):
                    ps = ps_proj.tile([128, 512], f32, tag="proj")
                    for nn in range(4):
                        jj = nn + 8
                        nc.tensor.matmul(
                            ps[:],
                            wo_b[:, D * nn + 128 * dt:D * nn + 128 * (dt + 1)],
                            aout_sb[:, 512 * jj:512 * (jj + 1)],
                            start=(nn == 0), stop=(nn == 3))
                    nc.vector.tensor_add(yhalf[:, 512 * dt:512 * (dt + 1)],
                                         ps[:], yhalf[:, 512 * dt:512 * (dt + 1)])

                aout_load(3)
                # pass3: e3's features, final add + stream y.T out
                for dt in range(DCH):
                    ps = ps_proj.tile([128, 512], f32, tag="proj")
                    for nn in range(4):
                        jj = nn + 12
                        nc.tensor.matmul(
                            ps[:],
                            wo_b[:, D * (nn + 4) + 128 * dt:D * (nn + 4) + 128 * (dt + 1)],
                            aout_sb[:, 512 * jj:512 * (jj + 1)],
                            start=(nn == 0), stop=(nn == 3))
                    yv = yo.tile([128, 512], f32, tag="y")
                    nc.vector.tensor_add(yv[:], ps[:],
                                         yhalf[:, 512 * dt:512 * (dt + 1)])
                    nc.sync.dma_start(
                        out=yT_ext[:, 512 * dt:512 * (dt + 1)],
                        in_=yv[:])

    nc.compile()
    return nc


def _host_inputs(x, wq, wk, wv, wo):
    bf = ml_dtypes.bfloat16

    # xT per batch: [128, 16*2048]; img[p, 2048i + t] = x[b, t, 128i + p]
    xT = []
    for b in range(B):
        t = x[b].T.reshape(DCH, 128, L).transpose(1, 0, 2).reshape(128, DCH * L)
        xT.append(np.ascontiguousarray(t).astype(bf))

    # wq per group g: e-major image; col 2048e + 128i + r ; row = qrow(g,e,r)
    wqT = []
    for g in range(4):
        img = np.empty((128, 4 * D), np.float32)
        for e in range(4):
            rows = np.concatenate([
                np.arange(64) + 64 * (8 * g + e),
                np.arange(64) + 64 * (8 * g + 4 + e)])
            Wsel = wq[rows, :]                      # [128, 2048]
            blk = Wsel.T.reshape(DCH, 128, 128)     # [i, p, r]
            img[:, 2048 * e:2048 * (e + 1)] = \
                blk.transpose(1, 0, 2).reshape(128, 2048)
        wqT.append(np.ascontiguousarray(img).astype(bf))

    # wkv per group g: t-major (K then V); rows 128g..128g+128 of wk/wv
    wkvT = []
    for g in range(4):
        img = np.empty((128, 2 * D), np.float32)
        for t, W in enumerate((wk, wv)):
            Wsel = W[128 * g:128 * (g + 1), :]
            blk = Wsel.T.reshape(DCH, 128, 128)
            img[:, 2048 * t:2048 * (t + 1)] = \
                blk.transpose(1, 0, 2).reshape(128, 2048)
        wkvT.append(np.ascontiguousarray(img).astype(bf))

    # wo (shared): col 2048jj + eo, row p; f(jj=4e+r, p)
    forder = np.empty(2048, np.int64)
    for jj in range(16):
        e, r = jj // 4, jj % 4
        p = np.arange(128)
        head = np.where(p < 64, 8 * r + e, 8 * r + 4 + e)
        forder[128 * jj:128 * (jj + 1)] = 64 * head + (p % 64)
    Wsel = wo[:, forder]                            # [2048 eo, 2048 f]
    woT = np.ascontiguousarray(
        Wsel.T.reshape(16, 128, D).transpose(1, 0, 2).reshape(128, 16 * D)
    ).astype(bf)

    # rope tables [128, 2048]: row r -> dim d = r%64
    freqs = 1.0 / (THETA ** (np.arange(0, HEAD_DIM, 2, dtype=np.float32) / HEAD_DIM))
    pos = np.arange(L, dtype=np.float32)
    ph = np.outer(freqs, pos)                       # [32, L]
    cos64 = np.concatenate([np.cos(ph), np.cos(ph)], axis=0)   # [64, L]
    sin64 = np.concatenate([np.sin(ph), np.sin(ph)], axis=0)
    sgn = np.where(np.arange(64) < 32, -1.0, 1.0)[:, None].astype(np.float32)
    cosT = np.concatenate([cos64, cos64], axis=0).astype(bf)
    sinT = np.concatenate([sin64 * sgn, sin64 * sgn], axis=0).astype(bf)

    q_idx = np.arange(CH)
    k_idx = np.arange(128)
    m0 = np.where(k_idx[:, None] <= q_idx[None, :], 1.0, 0.0)
    m1 = np.where(k_idx[:, None] + 128 <= q_idx[None, :], 1.0, 0.0)
    mask = np.concatenate([m0, m1], axis=1).astype(bf)

    in_maps = []
    for c in range(N_CORES):
        b, g = c // 4, c % 4
        in_maps.append({
            "xT": xT[b], "wqT": wqT[g], "wkvT": wkvT[g], "woT": woT,
            "cosT": cosT, "sinT": sinT, "maskT": mask,
        })
    return in_maps


def kernel(x, wq, wk, wv, wo):
    from concourse.bass_utils import run_bass_kernel_spmd

    if "nc" not in _BUILT:
        _BUILT["nc"] = _build_nc()
    nc = _BUILT["nc"]

    in_maps = _host_inputs(np.asarray(x), np.asarray(wq), np.asarray(wk),
                           np.asarray(wv), np.asarray(wo))
    trace = bool(os.environ.get("BASS_KERNEL_TRACE"))
    res = run_bass_kernel_spmd(nc, in_maps, core_ids=list(range(N_CORES)),
                               trace=trace)
    kernel.last_exec_time_ns = res.exec_time_ns
    kernel.last_results = res

    y = np.empty((B, L, D), dtype=np.float32)
    for c in range(N_CORES):
        arr = res.results[c]["yT"].reshape(128, DCH, 2, CH)
        for b2 in range(2):
            y[b2, CH * c:CH * (c + 1), :] = \
                arr[:, :, b2, :].transpose(2, 1, 0).reshape(CH, D)
    return y


# revision 12
# speedup vs baseline: 1.1429x; 1.0932x over previous
"""GQA attention with RoPE on 8 TRN2 NeuronCores (Bass/Tile, bf16).

Sharding: head + batch tensor parallel.
  - Core c = (b=c//4, g=c%4) owns batch b, query heads 8g..8g+7 and kv
    heads {2g, 2g+1}. Projections, RoPE and causal attention for those
    heads run with ZERO cross-core communication.
  - e-tiles pair heads (8g+e, 8g+4+e) so one 128-row tile holds a
    (kv 2g, kv 2g+1) head pair; scores run as two 64x128 row-tiled
    matmuls (K.T stationary, feature-major Q moving) producing S.T so
    softmax'd probs feed AV without transposes; denominators come free
    from a ones column appended to V.
  - A tiny warm-up AllToAll at kernel start absorbs cross-core launch
    skew on the collective queue, so real AllToAlls rendezvous fast.
    Each e-tile's AllToAll triggers as soon as its data is complete
    (e0+e1 share one buffer -> after e1; e2 after e2; e3 after e3),
    overlapping the exchange with the next e-tile's attention.
  - Fillers: e0 carries q_proj(1)+q_proj(2), e1 carries q_proj(3) (so
    the x/wq SBUF pool frees after e1 and wo/aout load during e2);
    o-proj feature chunks fill e2 (dt 0-5) and e3 (dt 6-15); the tail
    is pass2 (e2's features, overlapping the last AllToAll) + pass3.
"""
import os
import numpy as np
import ml_dtypes

N_CORES = 8
B, L, D = 2, 2048, 2048
N_HEADS, KV_HEADS, HEAD_DIM = 32, 8, 64
THETA = 10000.0
DCH = D // 128            # 16 contraction chunks
NKB = L // 128            # 16 key blocks
CH = 256                  # q chunk
NCH = L // CH             # 8 q chunks
TSH = L // 4              # 512 output tokens per core

_BUILT = {}


def _build_nc():
    import concourse.bacc as bacc
    import concourse.tile as tile
    from concourse import mybir
    from concourse.masks import make_identity

    f32 = mybir.dt.float32
    bf16 = mybir.dt.bfloat16

    nc = bacc.Bacc("TRN2", target_bir_lowering=False, debug=False,
                   num_devices=N_CORES)

    xT_ext = nc.dram_tensor("xT", [128, DCH * L], bf16, kind="ExternalInput")
    wqT_ext = nc.dram_tensor("wqT", [128, 4 * D], bf16, kind="ExternalInput")
    wkvT_ext = nc.dram_tensor("wkvT", [128, 2 * D], bf16, kind="ExternalInput")
    woT_ext = nc.dram_tensor("woT", [128, 16 * D], bf16, kind="ExternalInput")
    cos_ext = nc.dram_tensor("cosT", [128, L], bf16, kind="ExternalInput")
    sin_ext = nc.dram_tensor("sinT", [128, L], bf16, kind="ExternalInput")
    mask_ext = nc.dram_tensor("maskT", [128, 2 * CH], bf16, kind="ExternalInput")
    yT_ext = nc.dram_tensor("yT", [128, 16 * TSH], f32, kind="ExternalOutput")

    rg = [list(range(N_CORES))]

    with tile.TileContext(nc) as tc:
        with tc.tile_pool(name="dram", bufs=1, space="DRAM") as dram, \
             tc.tile_pool(name="const", bufs=1) as const, \
             tc.tile_pool(name="kv", bufs=1) as kv, \
             tc.tile_pool(name="att", bufs=1) as att, \
             tc.tile_pool(name="ptp", bufs=3) as ptp, \
             tc.tile_pool(name="rope", bufs=4) as rp, \
             tc.tile_pool(name="nrm", bufs=3) as nrm, \
             tc.tile_pool(name="ps_proj", bufs=2, space="PSUM") as ps_proj, \
             tc.tile_pool(name="ps_sc", bufs=2, space="PSUM") as ps_sc, \
             tc.tile_pool(name="ps_av", bufs=2, space="PSUM") as ps_av:

            warm_i = dram.tile([8, 16], bf16, tag="warm_i")
            warm_o = dram.tile([8, 16], bf16, tag="warm_o")
            ain01 = dram.tile([1024, 2 * CH], bf16, tag="ain01")
            ain2 = dram.tile([1024, CH], bf16, tag="ain2")
            ain3 = dram.tile([1024, CH], bf16, tag="ain3")
            aout01 = dram.tile([1024, 2 * CH], bf16, tag="aout01")
            aout2 = dram.tile([1024, CH], bf16, tag="aout2")
            aout3 = dram.tile([1024, CH], bf16, tag="aout3")

            # Warm-up collective: rendezvous-only. All 8 cores meet here
            # at kernel start, so the launch skew is paid on the CC queue
            # while the compute engines stream phase 1, instead of inside
            # the first real AllToAll mid-kernel.
            nc.gpsimd.collective_compute(
                "AllToAll", mybir.AluOpType.bypass, replica_groups=rg,
                ins=[warm_i[:].opt()], outs=[warm_o[:].opt()])

            cos_sb = const.tile([128, L], bf16, tag="cos")
            sin_sb = const.tile([128, L], bf16, tag="sin")
            mask_sb = const.tile([128, 2 * CH], bf16, tag="mask")
            ident = const.tile([128, 128], bf16, tag="ident")
            # exp via int arithmetic (Schraudolph), 16-bit variant: the
            # bf16 bit pattern of exp(s/8) ~= int16(s*A16 + B16), written
            # straight into the bf16 prob tile (no separate copy op).
            expB = const.tile([128, 1024], f32, tag="expB")
            nc.any.memset(expB[:], float((127 << 7)) - 7.42)
            nc.gpsimd.dma_start(out=cos_sb[:], in_=cos_ext[:])
            nc.gpsimd.dma_start(out=sin_sb[:], in_=sin_ext[:])
            nc.gpsimd.dma_start(out=mask_sb[:], in_=mask_ext[:])
            make_identity(nc, ident[:])

            kT = kv.tile([128, L], bf16, tag="kT")
            vT = kv.tile([128, L], bf16, tag="vT")
            vones = [kv.tile([128, NKB * 65], bf16, name=f"vo{h}", tag=f"vo{h}")
                     for h in range(2)]
            qT = [kv.tile([128, L], bf16, name=f"qT{e}", tag=f"qT{e}")
                  for e in range(4)]
            for h in range(2):
                nc.any.memset(vones[h][:], 1.0)

            def rope_to(ps, out_slice, cslice):
                """RoPE a [128, 512] feature-major psum chunk into bf16 sbuf.
                Rows r: dim d = r%64; rotate-half via partition-shifted psum
                reads; sinT has the sign baked in host-side."""
                t1 = rp.tile([128, 512], bf16, tag="t1")
                nc.vector.tensor_mul(t1[:], ps[:], cos_sb[:, cslice])
                t2 = rp.tile([128, 512], bf16, tag="t2")
                for hh in range(2):
                    b0 = 64 * hh
                    nc.vector.tensor_mul(t2[b0:b0 + 32, :],
                                         ps[b0 + 32:b0 + 64, :],
                                         sin_sb[b0:b0 + 32, cslice])
                    nc.vector.tensor_mul(t2[b0 + 32:b0 + 64, :],
                                         ps[b0:b0 + 32, :],
                                         sin_sb[b0 + 32:b0 + 64, cslice])
                nc.vector.tensor_add(out_slice, t1[:], t2[:])

            # shared attention helpers (defined once, used for all e-tiles)
            def ptcol(c, kb, hh):
                """pt column of key block kb, head-half hh for chunk c.
                Full groups of 4 blocks hold [hh0 x4 | hh1 x4]; a 2-block
                tail packs [hh0 x2 | hh1 x2] so ONE exp covers both."""
                nkb = 2 * (c + 1)
                k4, s = kb // 4, kb % 4
                if 4 * k4 + 4 <= nkb:
                    return 2048 * k4 + 1024 * hh + 256 * s
                return 2048 * k4 + 512 * hh + 256 * s

            EXPA16 = float(128.0 / np.log(2.0) * 0.125)

            def emit_exp(dst, ssp, on_dve):
                if not on_dve:
                    nc.scalar.activation(
                        dst, ssp, mybir.ActivationFunctionType.Exp,
                        scale=0.125)
                    return
                # DVE offload: bf16 bits of exp(s/8) ~= int16(s*A16 + B16);
                # ~3% sawtooth, fine at rel-err budget; keeps ACT from
                # rate-limiting the attention pipeline, and the int16
                # result IS the bf16 prob (one DVE op, no copy).
                n = ssp.shape[1]
                nc.vector.scalar_tensor_tensor(
                    dst.bitcast(mybir.dt.int16), ssp, EXPA16, expB[:, 0:n],
                    mybir.AluOpType.mult, mybir.AluOpType.add)

            def scores_chunk(e, c):
                """S.T for q chunk c, both head-halves -> exp'd probs."""
                nkb = 2 * (c + 1)
                pt = ptp.tile([128, 4 * 2048], bf16, tag="pt",
                              name=f"pt_{e}_{c}")
                for k4 in range(nkb // 4):
                    for hh in range(2):
                        h0 = 64 * hh
                        ssp = ps_sc.tile([128, 1024], f32, tag="sc")
                        for s in range(4):
                            kb = 4 * k4 + s
                            nc.tensor.matmul(
                                ssp[:, 256 * s:256 * (s + 1)],
                                kT[h0:h0 + 64, 128 * kb:128 * (kb + 1)],
                                qT[e][h0:h0 + 64, CH * c:CH * (c + 1)],
                                start=True, stop=True)
                        emit_exp(
                            pt[:, 2048 * k4 + 1024 * hh:
                               2048 * k4 + 1024 * (hh + 1)],
                            ssp[:], on_dve=(hh == 1 and c % 2 == 1))
                if nkb % 4:
                    k4 = nkb // 4
                    ssp = ps_sc.tile([128, 1024], f32, tag="sc")
                    for hh in range(2):
                        h0 = 64 * hh
                        for s in range(2):
                            kb = 4 * k4 + s
                            nc.tensor.matmul(
                                ssp[:, 256 * (2 * hh + s):
                                    256 * (2 * hh + s + 1)],
                                kT[h0:h0 + 64, 128 * kb:128 * (kb + 1)],
                                qT[e][h0:h0 + 64, CH * c:CH * (c + 1)],
                                start=True, stop=True)
                    nc.scalar.activation(
                        pt[:, 2048 * k4:2048 * k4 + 1024], ssp[:],
                        mybir.ActivationFunctionType.Exp, scale=0.125)
                for hh in range(2):
                    c0 = ptcol(c, 2 * c, hh)
                    nc.vector.tensor_mul(pt[:, c0:c0 + 512],
                                         pt[:, c0:c0 + 512], mask_sb[:])
                return pt

            def av_chunk(e, c, hh, pt, avt, _ptmem={}):
                if pt is not None:
                    _ptmem['pt'] = pt
                pt = _ptmem['pt']
                nkb = 2 * (c + 1)
                av = avt[:, 256 * hh:256 * (hh + 1)]
                for kb in range(nkb):
                    c0 = ptcol(c, kb, hh)
                    nc.tensor.matmul(
                        av,
                        vones[hh][:, 65 * kb:65 * kb + 65],
                        pt[:, c0:c0 + 256],
                        start=(kb == 0), stop=(kb == nkb - 1))

            def av_norm(avt, an_t):
                """Normalize both head-halves of one chunk: one denominator
                copy + reciprocal + broadcast (recip from PSUM is silently
                wrong on HW, so stage through SBUF)."""
                lrow = nrm.tile([1, 512], f32, tag="lrow")
                nc.vector.tensor_copy(lrow[:], avt[64:65, :])
                linv = nrm.tile([1, 512], f32, tag="linv")
                nc.vector.reciprocal_approx_fast(out=linv[:], in_=lrow[:])
                bcs = nrm.tile([64, 512], f32, tag="bcs")
                nc.gpsimd.partition_broadcast(bcs[:], linv[0:1, :],
                                              channels=64)
                for hh in range(2):
                    nc.vector.tensor_mul(
                        an_t[64 * hh:64 * hh + 64, :],
                        avt[0:64, 256 * hh:256 * (hh + 1)],
                        bcs[:, 256 * hh:256 * (hh + 1)])

            def att_etile(e, fills_at=None):
                """Attention for e-tile e; fills_at maps chunk index ->
                list of thunks emitting PE filler work at that chunk."""
                fills_at = fills_at or {}
                ain_t = (ain01, ain01, ain2, ain3)[e]
                ecol = CH * e if e < 2 else 0
                dq = nc.gpsimd
                pts = {}

                def do_av(pc):
                    avt = ps_av.tile([65, 512], f32, tag="av")
                    an_t = nrm.tile([128, CH], bf16, tag="an")
                    av_chunk(e, pc, 0, pts.pop(pc), avt)
                    av_chunk(e, pc, 1, None, avt)
                    av_norm(avt, an_t)
                    dq.dma_start(
                        out=ain_t[128 * pc:128 * (pc + 1), ecol:ecol + CH],
                        in_=an_t[:])

                for c in range(NCH):
                    pts[c] = scores_chunk(e, c)
                    for th in fills_at.get(c, ()):
                        th()
                    if c >= 2:
                        do_av(c - 2)   # 2-chunk lag hides exp+mask latency
                do_av(NCH - 2)
                do_av(NCH - 1)

            # -------- phase 1: loads + K/V/Q projections (local) --------
            with tc.tile_pool(name="xw", bufs=1) as xw:
                xT_sb = xw.tile([128, DCH * L], bf16, tag="xT")
                wq_sb = xw.tile([128, 4 * D], bf16, tag="wq")
                wkv_sb = xw.tile([128, 2 * D], bf16, tag="wkv")
                # wkv first (KV proj starts earliest), xT split across both
                # queues, wq (e-major host layout) on gpsimd
                nc.sync.dma_start(out=wkv_sb[:], in_=wkvT_ext[:])
                for i in range(DCH):
                    q = nc.sync if i % 2 == 0 else nc.gpsimd
                    q.dma_start(out=xT_sb[:, L * i:L * (i + 1)],
                                in_=xT_ext[:, L * i:L * (i + 1)])
                nc.gpsimd.dma_start(out=wq_sb[:], in_=wqT_ext[:])

                def kv_proj(t, dst_rope):
                    for tc4 in range(4):
                        # borrow idle score-psum slots so 4 projection groups
                        # accumulate concurrently while xT chunks stream in
                        pool = ps_proj if tc4 % 2 == 0 else ps_sc
                        ps = pool.tile([128, 512], f32,
                                       tag="proj" if tc4 % 2 == 0 else "sc")
                        for i in range(DCH):
                            nc.tensor.matmul(
                                ps[:],
                                wkv_sb[:, 2048 * t + 128 * i:2048 * t + 128 * (i + 1)],
                                xT_sb[:, L * i + 512 * tc4:L * i + 512 * (tc4 + 1)],
                                start=(i == 0), stop=(i == DCH - 1))
                        sl = slice(512 * tc4, 512 * (tc4 + 1))
                        if dst_rope:
                            rope_to(ps[:], kT[:, sl], sl)
                        else:
                            nc.scalar.copy(vT[:, sl], ps[:])

                def q_proj(e, tc4):
                    ps = ps_proj.tile([128, 512], f32, tag="proj")
                    for i in range(DCH):
                        nc.tensor.matmul(
                            ps[:],
                            wq_sb[:, 2048 * e + 128 * i:2048 * e + 128 * (i + 1)],
                            xT_sb[:, L * i + 512 * tc4:L * i + 512 * (tc4 + 1)],
                            start=(i == 0), stop=(i == DCH - 1))
                    sl = slice(512 * tc4, 512 * (tc4 + 1))
                    rope_to(ps[:], qT[e][:, sl], sl)

                kv_proj(0, True)   # K
                kv_proj(1, False)  # V
                # V transposes: [128 dims, 128 keys] -> [128 keys, 128 dims]
                for kb in range(NKB):
                    tp = ps_proj.tile([128, 128], bf16, tag="proj")
                    nc.tensor.transpose(tp[:], vT[:, 128 * kb:128 * (kb + 1)],
                                        ident[:])
                    nc.vector.tensor_copy(vones[0][:, 65 * kb:65 * kb + 64],
                                          tp[:, 0:64])
                    nc.vector.tensor_copy(vones[1][:, 65 * kb:65 * kb + 64],
                                          tp[:, 64:128])
                for tc4 in range(4):
                    q_proj(0, tc4)

                # -------- phase 2: attention e0/e1 (q fills drain xw) ----
                att_etile(0, {2: [lambda: q_proj(1, 0), lambda: q_proj(1, 1)],
                              3: [lambda: q_proj(1, 2), lambda: q_proj(1, 3)],
                              4: [lambda: q_proj(2, 0), lambda: q_proj(2, 1)],
                              5: [lambda: q_proj(2, 2), lambda: q_proj(2, 3)]})
                att_etile(1, {4: [lambda: q_proj(3, 0)],
                              5: [lambda: q_proj(3, 1)],
                              6: [lambda: q_proj(3, 2)],
                              7: [lambda: q_proj(3, 3)]})
                # e0+e1 head-sharded outputs are complete: exchange now,
                # overlapping att(e2). With the warm-up A2A having absorbed
                # launch skew, rendezvous here is short.
                nc.gpsimd.collective_compute(
                    "AllToAll", mybir.AluOpType.bypass, replica_groups=rg,
                    ins=[ain01[:].opt()], outs=[aout01[:].opt()])

            # -------- phase 3: wo/aout staging + e2/e3 with o-proj fill --
            with tc.tile_pool(name="wop", bufs=1) as wop, \
                 tc.tile_pool(name="yo", bufs=4) as yo:
                wo_sb = wop.tile([128, 8 * D], bf16, tag="wo")
                aout_sb = wop.tile([128, 16 * TSH], bf16, tag="aout")
                yhalf = wop.tile([128, 16 * 512], bf16, tag="yhalf")
                for jj in range(8):
                    nc.sync.dma_start(out=wo_sb[:, D * jj:D * (jj + 1)],
                                      in_=woT_ext[:, D * jj:D * (jj + 1)])

                # aout_sb col 512*(4e+gg) = [b0 256 | b1 256] for (e, gg)
                def aout_load(e):
                    at = (aout01, aout01, aout2, aout3)[e]
                    ecol = CH * e if e < 2 else 0
                    for j in range(8):
                        b2, gg = j // 4, j % 4
                        dst = 512 * (4 * e + gg) + 256 * b2
                        nc.sync.dma_start(
                            out=aout_sb[:, dst:dst + 256],
                            in_=at[128 * j:128 * (j + 1), ecol:ecol + CH])

                aout_load(0)
                aout_load(1)

                def oproj_dt(dt):
                    """y.T chunk dt from features jj 0-7 (e0+e1) -> yhalf."""
                    ps = ps_proj.tile([128, 512], f32, tag="proj")
                    for jj in range(8):
                        nc.tensor.matmul(
                            ps[:],
                            wo_sb[:, D * jj + 128 * dt:D * jj + 128 * (dt + 1)],
                            aout_sb[:, 512 * jj:512 * (jj + 1)],
                            start=(jj == 0), stop=(jj == 7))
                    nc.scalar.copy(yhalf[:, 512 * dt:512 * (dt + 1)], ps[:])

                def op2(d0):
                    return lambda: (oproj_dt(d0), oproj_dt(d0 + 1))

                att_etile(2, {5: [op2(0)], 6: [op2(2)], 7: [op2(4)]})
                nc.gpsimd.collective_compute(
                    "AllToAll", mybir.AluOpType.bypass, replica_groups=rg,
                    ins=[ain2[:].opt()], outs=[aout2[:].opt()])
                aout_load(2)

                att_etile(3, {1: [op2(6)], 2: [op2(8)], 3: [op2(10)],
                              4: [op2(12)], 5: [op2(14)]})
                nc.gpsimd.collective_compute(
                    "AllToAll", mybir.AluOpType.bypass, replica_groups=rg,
                    ins=[ain3[:].opt()], outs=[aout3[:].opt()])

                # wo jj 8-15 reuses wo_sb's slot once the e2/e3 o-proj
                # fills have drained it (late in e3)
                wo_b = wop.tile([128, 8 * D], bf16, tag="wo")
                for jj in range(8):
                    nc.sync.dma_start(out=wo_b[:, D * jj:D * (jj + 1)],
                                      in_=woT_ext[:, D * (jj + 8):D * (jj + 9)])

                # pass2: e2's features accumulate while the e3 AllToAll runs
                for dt in range(DCH):
                    ps = ps_proj.tile([128, 512], f32, tag="proj")
                    for nn in range(4):
                        jj = nn + 8
                        nc.tensor.matmul(
                            ps[:],
                            wo_b[:, D * nn + 128 * dt:D * nn + 128 * (dt + 1)],
                            aout_sb[:, 512 * jj:512 * (jj + 1)],
                            start=(nn == 0), stop=(nn == 3))
                    nc.vector.tensor_add(yhalf[:, 512 * dt:512 * (dt + 1)],
                                         ps[:], yhalf[:, 512 * dt:512 * (dt + 1)])

                aout_load(3)
                # pass3: e3's features, final add + stream y.T out
                for dt in range(DCH):
                    ps = ps_proj.tile([128, 512], f32, tag="proj")
                    for nn in range(4):
                        jj = nn + 12
                        nc.tensor.matmul(
                            ps[:],
                            wo_b[:, D * (nn + 4) + 128 * dt:D * (nn + 4) + 128 * (dt + 1)],
                            aout_sb[:, 512 * jj:512 * (jj + 1)],
                            start=(nn == 0), stop=(nn == 3))
                    yv = yo.tile([128, 512], f32, tag="y")
                    nc.vector.tensor_add(yv[:], ps[:],
                                         yhalf[:, 512 * dt:512 * (dt + 1)])
                    nc.sync.dma_start(
                        out=yT_ext[:, 512 * dt:512 * (dt + 1)],
                        in_=yv[:])

    nc.compile()
    return nc


def _host_inputs(x, wq, wk, wv, wo):
    bf = ml_dtypes.bfloat16

    # xT per batch: [128, 16*2048]; img[p, 2048i + t] = x[b, t, 128i + p]
    xT = []
    for b in range(B):
        t = x[b].T.reshape(DCH, 128, L).transpose(1, 0, 2).reshape(128, DCH * L)
        xT.append(np.ascontiguousarray(t).astype(bf))

    # wq per group g: e-major image; col 2048e + 128i + r ; row = qrow(g,e,r)
    wqT = []
    for g in range(4):
        img = np.empty((128, 4 * D), np.float32)
        for e in range(4):
            rows = np.concatenate([
                np.arange(64) + 64 * (8 * g + e),
                np.arange(64) + 64 * (8 * g + 4 + e)])
            Wsel = wq[rows, :]                      # [128, 2048]
            blk = Wsel.T.reshape(DCH, 128, 128)     # [i, p, r]
            img[:, 2048 * e:2048 * (e + 1)] = \
                blk.transpose(1, 0, 2).reshape(128, 2048)
        wqT.append(np.ascontiguousarray(img).astype(bf))

    # wkv per group g: t-major (K then V); rows 128g..128g+128 of wk/wv
    wkvT = []
    for g in range(4):
        img = np.empty((128, 2 * D), np.float32)
        for t, W in enumerate((wk, wv)):
            Wsel = W[128 * g:128 * (g + 1), :]
            blk = Wsel.T.reshape(DCH, 128, 128)
            img[:, 2048 * t:2048 * (t + 1)] = \
                blk.transpose(1, 0, 2).reshape(128, 2048)
        wkvT.append(np.ascontiguousarray(img).astype(bf))

    # wo (shared): col 2048jj + eo, row p; f(jj=4e+r, p)
    forder = np.empty(2048, np.int64)
    for jj in range(16):
        e, r = jj // 4, jj % 4
        p = np.arange(128)
        head = np.where(p < 64, 8 * r + e, 8 * r + 4 + e)
        forder[128 * jj:128 * (jj + 1)] = 64 * head + (p % 64)
    Wsel = wo[:, forder]                            # [2048 eo, 2048 f]
    woT = np.ascontiguousarray(
        Wsel.T.reshape(16, 128, D).transpose(1, 0, 2).reshape(128, 16 * D)
    ).astype(bf)

    # rope tables [128, 2048]: row r -> dim d = r%64
    freqs = 1.0 / (THETA ** (np.arange(0, HEAD_DIM, 2, dtype=np.float32) / HEAD_DIM))
    pos = np.arange(L, dtype=np.float32)
    ph = np.outer(freqs, pos)                       # [32, L]
    cos64 = np.concatenate([np.cos(ph), np.cos(ph)], axis=0)   # [64, L]
    sin64 = np.concatenate([np.sin(ph), np.sin(ph)], axis=0)
    sgn = np.where(np.arange(64) < 32, -1.0, 1.0)[:, None].astype(np.float32)
    cosT = np.concatenate([cos64, cos64], axis=0).astype(bf)
    sinT = np.concatenate([sin64 * sgn, sin64 * sgn], axis=0).astype(bf)

    q_idx = np.arange(CH)
    k_idx = np.arange(128)
    m0 = np.where(k_idx[:, None] <= q_idx[None, :], 1.0, 0.0)
    m1 = np.where(k_idx[:, None] + 128 <= q_idx[None, :], 1.0, 0.0)
    mask = np.concatenate([m0, m1], axis=1).astype(bf)

    in_maps = []
    for c in range(N_CORES):
        b, g = c // 4, c % 4
        in_maps.append({
            "xT": xT[b], "wqT": wqT[g], "wkvT": wkvT[g], "woT": woT,
            "cosT": cosT, "sinT": sinT, "maskT": mask,
        })
    return in_maps


def kernel(x, wq, wk, wv, wo):
    from concourse.bass_utils import run_bass_kernel_spmd

    if "nc" not in _BUILT:
        _BUILT["nc"] = _build_nc()
    nc = _BUILT["nc"]

    in_maps = _host_inputs(np.asarray(x), np.asarray(wq), np.asarray(wk),
                           np.asarray(wv), np.asarray(wo))
    trace = bool(os.environ.get("BASS_KERNEL_TRACE"))
    res = run_bass_kernel_spmd(nc, in_maps, core_ids=list(range(N_CORES)),
                               trace=trace)
    kernel.last_exec_time_ns = res.exec_time_ns
    kernel.last_results = res

    y = np.empty((B, L, D), dtype=np.float32)
    for c in range(N_CORES):
        arr = res.results[c]["yT"].reshape(128, DCH, 2, CH)
        for b2 in range(2):
            y[b2, CH * c:CH * (c + 1), :] = \
                arr[:, :, b2, :].transpose(2, 1, 0).reshape(CH, D)
    return y


# revision 17
# speedup vs baseline: 1.2305x; 1.0767x over previous
"""GQA attention with RoPE on 8 TRN2 NeuronCores (Bass/Tile, bf16).

Sharding: head + batch tensor parallel.
  - Core c = (b=c//4, g=c%4) owns batch b, query heads 8g..8g+7 and kv
    heads {2g, 2g+1}. Projections, RoPE and causal attention for those
    heads run with ZERO cross-core communication.
  - e-tiles pair heads (8g+e, 8g+4+e) so one 128-row tile holds a
    (kv 2g, kv 2g+1) head pair; scores run as two 64x128 row-tiled
    matmuls (K.T stationary, feature-major Q moving) producing S.T so
    softmax'd probs feed AV without transposes; denominators come free
    from a ones column appended to V.
  - A tiny warm-up AllToAll at kernel start absorbs cross-core launch
    skew on the collective queue, so real AllToAlls rendezvous fast.
    Each e-tile's AllToAll triggers as soon as its data is complete
    (e0+e1 share one buffer -> after e1; e2 after e2; e3 after e3),
    overlapping the exchange with the next e-tile's attention.
  - Fillers: e0 carries q_proj(1)+q_proj(2), e1 carries q_proj(3) (so
    the x/wq SBUF pool frees after e1 and wo/aout load during e2);
    o-proj feature chunks fill e2 (dt 0-5) and e3 (dt 6-15); the tail
    is pass2 (e2's features, overlapping the last AllToAll) + pass3.
"""
import os
import numpy as np
import ml_dtypes

N_CORES = 8
B, L, D = 2, 2048, 2048
N_HEADS, KV_HEADS, HEAD_DIM = 32, 8, 64
THETA = 10000.0
DCH = D // 128            # 16 contraction chunks
NKB = L // 128            # 16 key blocks
CH = 256                  # q chunk
NCH = L // CH             # 8 q chunks
TSH = L // 4              # 512 output tokens per core

_BUILT = {}


def _build_nc():
    import concourse.bacc as bacc
    import concourse.tile as tile
    from concourse import mybir
    from concourse.masks import make_identity

    f32 = mybir.dt.float32
    bf16 = mybir.dt.bfloat16

    nc = bacc.Bacc("TRN2", target_bir_lowering=False, debug=False,
                   num_devices=N_CORES)

    xT_ext = nc.dram_tensor("xT", [128, DCH * L], bf16, kind="ExternalInput")
    wqT_ext = nc.dram_tensor("wqT", [128, 4 * D], bf16, kind="ExternalInput")
    wkvT_ext = nc.dram_tensor("wkvT", [128, 2 * D], bf16, kind="ExternalInput")
    woT_ext = nc.dram_tensor("woT", [128, 16 * D], bf16, kind="ExternalInput")
    cos_ext = nc.dram_tensor("cosT", [128, L], bf16, kind="ExternalInput")
    sin_ext = nc.dram_tensor("sinT", [128, L], bf16, kind="ExternalInput")
    mask_ext = nc.dram_tensor("maskT", [128, 2 * CH], bf16, kind="ExternalInput")
    yT_ext = nc.dram_tensor("yT", [128, 16 * TSH], f32, kind="ExternalOutput")

    rg = [list(range(N_CORES))]

    with tile.TileContext(nc) as tc:
        with tc.tile_pool(name="dram", bufs=1, space="DRAM") as dram, \
             tc.tile_pool(name="const", bufs=1) as const, \
             tc.tile_pool(name="kv", bufs=1) as kv, \
             tc.tile_pool(name="att", bufs=1) as att, \
             tc.tile_pool(name="ptp", bufs=3) as ptp, \
             tc.tile_pool(name="rope", bufs=4) as rp, \
             tc.tile_pool(name="nrm", bufs=3) as nrm, \
             tc.tile_pool(name="ps_proj", bufs=2, space="PSUM") as ps_proj, \
             tc.tile_pool(name="ps_sc", bufs=2, space="PSUM") as ps_sc, \
             tc.tile_pool(name="ps_av", bufs=2, space="PSUM") as ps_av:

            warm_i = dram.tile([8, 16], bf16, tag="warm_i")
            warm_o = dram.tile([8, 16], bf16, tag="warm_o")
            ain01 = dram.tile([1024, 2 * CH], bf16, tag="ain01")
            ain2 = dram.tile([1024, CH], bf16, tag="ain2")
            ain3 = dram.tile([1024, CH], bf16, tag="ain3")
            aout01 = dram.tile([1024, 2 * CH], bf16, tag="aout01")
            aout2 = dram.tile([1024, CH], bf16, tag="aout2")
            aout3 = dram.tile([1024, CH], bf16, tag="aout3")

            # Warm-up collective: rendezvous-only. All 8 cores meet here
            # at kernel start, so the launch skew is paid on the CC queue
            # while the compute engines stream phase 1, instead of inside
            # the first real AllToAll mid-kernel.
            nc.gpsimd.collective_compute(
                "AllToAll", mybir.AluOpType.bypass, replica_groups=rg,
                ins=[warm_i[:].opt()], outs=[warm_o[:].opt()])

            cos_sb = const.tile([128, L], bf16, tag="cos")
            sin_sb = const.tile([128, L], bf16, tag="sin")
            mask_sb = const.tile([128, 2 * CH], bf16, tag="mask")
            ident = const.tile([128, 128], bf16, tag="ident")
            # exp via int arithmetic (Schraudolph), 16-bit variant: the
            # bf16 bit pattern of exp(s/8) ~= int16(s*A16 + B16), written
            # straight into the bf16 prob tile (no separate copy op).
            expB = const.tile([128, 1024], f32, tag="expB")
            nc.any.memset(expB[:], float((127 << 7)) - 7.42)
            nc.gpsimd.dma_start(out=cos_sb[:], in_=cos_ext[:])
            nc.gpsimd.dma_start(out=sin_sb[:], in_=sin_ext[:])
            nc.gpsimd.dma_start(out=mask_sb[:], in_=mask_ext[:])
            make_identity(nc, ident[:])

            kT = kv.tile([128, L], bf16, tag="kT")
            vT = kv.tile([128, L], bf16, tag="vT")
            vones = [kv.tile([128, NKB * 65], bf16, name=f"vo{h}", tag=f"vo{h}")
                     for h in range(2)]
            qT = [kv.tile([128, L], bf16, name=f"qT{e}", tag=f"qT{e}")
                  for e in range(4)]
            for h in range(2):
                nc.any.memset(vones[h][:], 1.0)

            def rope_to(ps, out_slice, cslice):
                """RoPE a [128, 512] feature-major psum chunk into bf16 sbuf.
                Rows r: dim d = r%64; rotate-half via partition-shifted psum
                reads; sinT has the sign baked in host-side."""
                t1 = rp.tile([128, 512], bf16, tag="t1")
                nc.vector.tensor_mul(t1[:], ps[:], cos_sb[:, cslice])
                t2 = rp.tile([128, 512], bf16, tag="t2")
                for hh in range(2):
                    b0 = 64 * hh
                    nc.vector.tensor_mul(t2[b0:b0 + 32, :],
                                         ps[b0 + 32:b0 + 64, :],
                                         sin_sb[b0:b0 + 32, cslice])
                    nc.vector.tensor_mul(t2[b0 + 32:b0 + 64, :],
                                         ps[b0:b0 + 32, :],
                                         sin_sb[b0 + 32:b0 + 64, cslice])
                nc.vector.tensor_add(out_slice, t1[:], t2[:])

            # shared attention helpers (defined once, used for all e-tiles)
            def ptcol(c, kb, hh):
                """pt column of key block kb, head-half hh for chunk c.
                Full groups of 4 blocks hold [hh0 x4 | hh1 x4]; a 2-block
                tail packs [hh0 x2 | hh1 x2] so ONE exp covers both."""
                nkb = 2 * (c + 1)
                k4, s = kb // 4, kb % 4
                if 4 * k4 + 4 <= nkb:
                    return 2048 * k4 + 1024 * hh + 256 * s
                return 2048 * k4 + 512 * hh + 256 * s

            EXPA16 = float(128.0 / np.log(2.0) * 0.125)

            def emit_exp(dst, ssp, on_dve):
                if not on_dve:
                    nc.scalar.activation(
                        dst, ssp, mybir.ActivationFunctionType.Exp,
                        scale=0.125)
                    return
                # DVE offload: bf16 bits of exp(s/8) ~= int16(s*A16 + B16);
                # ~3% sawtooth, fine at rel-err budget; keeps ACT from
                # rate-limiting the attention pipeline, and the int16
                # result IS the bf16 prob (one DVE op, no copy).
                n = ssp.shape[1]
                nc.vector.scalar_tensor_tensor(
                    dst.bitcast(mybir.dt.int16), ssp, EXPA16, expB[:, 0:n],
                    mybir.AluOpType.mult, mybir.AluOpType.add)

            def scores_chunk(e, c):
                """S.T for q chunk c, both head-halves -> exp'd probs."""
                nkb = 2 * (c + 1)
                pt = ptp.tile([128, 4 * 2048], bf16, tag="pt",
                              name=f"pt_{e}_{c}")
                for k4 in range(nkb // 4):
                    for hh in range(2):
                        h0 = 64 * hh
                        ssp = ps_sc.tile([128, 1024], f32, tag="sc")
                        for s in range(4):
                            kb = 4 * k4 + s
                            nc.tensor.matmul(
                                ssp[:, 256 * s:256 * (s + 1)],
                                kT[h0:h0 + 64, 128 * kb:128 * (kb + 1)],
                                qT[e][h0:h0 + 64, CH * c:CH * (c + 1)],
                                start=True, stop=True)
                        emit_exp(
                            pt[:, 2048 * k4 + 1024 * hh:
                               2048 * k4 + 1024 * (hh + 1)],
                            ssp[:], on_dve=(hh == 1 and c % 2 == 1))
                if nkb % 4:
                    k4 = nkb // 4
                    ssp = ps_sc.tile([128, 1024], f32, tag="sc")
                    for hh in range(2):
                        h0 = 64 * hh
                        for s in range(2):
                            kb = 4 * k4 + s
                            nc.tensor.matmul(
                                ssp[:, 256 * (2 * hh + s):
                                    256 * (2 * hh + s + 1)],
                                kT[h0:h0 + 64, 128 * kb:128 * (kb + 1)],
                                qT[e][h0:h0 + 64, CH * c:CH * (c + 1)],
                                start=True, stop=True)
                    nc.scalar.activation(
                        pt[:, 2048 * k4:2048 * k4 + 1024], ssp[:],
                        mybir.ActivationFunctionType.Exp, scale=0.125)
                for hh in range(2):
                    c0 = ptcol(c, 2 * c, hh)
                    nc.vector.tensor_mul(pt[:, c0:c0 + 512],
                                         pt[:, c0:c0 + 512], mask_sb[:])
                return pt

            def av_chunk(e, c, hh, pt, avt, _ptmem={}):
                if pt is not None:
                    _ptmem['pt'] = pt
                pt = _ptmem['pt']
                nkb = 2 * (c + 1)
                av = avt[:, 256 * hh:256 * (hh + 1)]
                for kb in range(nkb):
                    c0 = ptcol(c, kb, hh)
                    nc.tensor.matmul(
                        av,
                        vones[hh][:, 65 * kb:65 * kb + 65],
                        pt[:, c0:c0 + 256],
                        start=(kb == 0), stop=(kb == nkb - 1))

            def av_norm(avt, an_t):
                """Normalize both head-halves of one chunk: one denominator
                copy + reciprocal + broadcast (recip from PSUM is silently
                wrong on HW, so stage through SBUF)."""
                lrow = nrm.tile([1, 512], f32, tag="lrow")
                nc.vector.tensor_copy(lrow[:], avt[64:65, :])
                linv = nrm.tile([1, 512], f32, tag="linv")
                nc.vector.reciprocal_approx_fast(out=linv[:], in_=lrow[:])
                bcs = nrm.tile([64, 512], f32, tag="bcs")
                nc.gpsimd.partition_broadcast(bcs[:], linv[0:1, :],
                                              channels=64)
                for hh in range(2):
                    nc.vector.tensor_mul(
                        an_t[64 * hh:64 * hh + 64, :],
                        avt[0:64, 256 * hh:256 * (hh + 1)],
                        bcs[:, 256 * hh:256 * (hh + 1)])

            def att_etile(e, fills_at=None):
                """Attention for e-tile e; fills_at maps chunk index ->
                list of thunks emitting PE filler work at that chunk."""
                fills_at = fills_at or {}
                ain_t = (ain01, ain01, ain2, ain3)[e]
                ecol = CH * e if e < 2 else 0
                dq = nc.gpsimd
                pts = {}

                def do_av(pc):
                    avt = ps_av.tile([65, 512], f32, tag="av")
                    an_t = nrm.tile([128, CH], bf16, tag="an")
                    av_chunk(e, pc, 0, pts.pop(pc), avt)
                    av_chunk(e, pc, 1, None, avt)
                    av_norm(avt, an_t)
                    dq.dma_start(
                        out=ain_t[128 * pc:128 * (pc + 1), ecol:ecol + CH],
                        in_=an_t[:])

                for c in range(NCH):
                    pts[c] = scores_chunk(e, c)
                    for th in fills_at.get(c, ()):
                        th()
                    if c >= 2:
                        do_av(c - 2)   # 2-chunk lag hides exp+mask latency
                do_av(NCH - 2)
                do_av(NCH - 1)

            # -------- phase 1: loads + K/V/Q projections (local) --------
            # xT streams in token-halves (h0 = tokens 0-1023 of every
            # feature chunk first), so the six h0 token-group projections
            # (K/V/Q0 groups 0,1) saturate the PE from ~13us, interleaved
            # per feature chunk to match DMA arrival order. The h1 groups
            # follow the same way; attention e0 starts ~60us earlier than
            # with whole-chunk streaming.
            with tc.tile_pool(name="xw", bufs=1) as xw:
                xT_sb = xw.tile([128, DCH * L], bf16, tag="xT")
                wq0_sb = xw.tile([128, D], bf16, tag="wq0")
                wqr_sb = xw.tile([128, 3 * D], bf16, tag="wqr")
                wkv_sb = xw.tile([128, 2 * D], bf16, tag="wkv")
                nc.sync.dma_start(out=wkv_sb[:], in_=wkvT_ext[:])
                nc.gpsimd.dma_start(out=wq0_sb[:], in_=wqT_ext[:, 0:D])
                for h in range(2):
                    for i in range(DCH):
                        q = nc.sync if i % 2 == 0 else nc.gpsimd
                        sl = slice(L * i + 1024 * h, L * i + 1024 * (h + 1))
                        q.dma_start(out=xT_sb[:, sl], in_=xT_ext[:, sl])
                nc.gpsimd.dma_start(out=wqr_sb[:], in_=wqT_ext[:, D:4 * D])

                def group_mm(ps, w_sb, wcol, tc4, i):
                    nc.tensor.matmul(
                        ps[:],
                        w_sb[:, wcol + 128 * i:wcol + 128 * (i + 1)],
                        xT_sb[:, L * i + 512 * tc4:L * i + 512 * (tc4 + 1)],
                        start=(i == 0), stop=(i == DCH - 1))

                def q_proj(e, tc4):
                    ps = ps_proj.tile([128, 512], f32, tag="proj")
                    w_sb = wq0_sb if e == 0 else wqr_sb
                    wcol = 0 if e == 0 else 2048 * (e - 1)
                    for i in range(DCH):
                        group_mm(ps, w_sb, wcol, tc4, i)
                    sl = slice(512 * tc4, 512 * (tc4 + 1))
                    rope_to(ps[:], qT[e][:, sl], sl)

                def kv_drain(t, tc4, ps):
                    sl = slice(512 * tc4, 512 * (tc4 + 1))
                    if t == 0:
                        rope_to(ps[:], kT[:, sl], sl)
                    else:
                        nc.scalar.copy(vT[:, sl], ps[:])

                # h0 token groups: K0 K1 V0 V1 Q00 Q01, interleaved per
                # feature chunk (6 matmuls per chunk arrival)
                h0_ps = []
                for gi, (pool, tag) in enumerate((
                        (ps_proj, "proj"), (ps_proj, "proj"),
                        (ps_sc, "sc"), (ps_sc, "sc"),
                        (ps_av, "av"), (ps_av, "av"))):
                    h0_ps.append(pool.tile([128, 512], f32, tag=tag,
                                           name=f"h0ps{gi}"))
                for i in range(DCH):
                    group_mm(h0_ps[0], wkv_sb, 0, 0, i)
                    group_mm(h0_ps[1], wkv_sb, 0, 1, i)
                    group_mm(h0_ps[2], wkv_sb, 2048, 0, i)
                    group_mm(h0_ps[3], wkv_sb, 2048, 1, i)
                    group_mm(h0_ps[4], wq0_sb, 0, 0, i)
                    group_mm(h0_ps[5], wq0_sb, 0, 1, i)
                kv_drain(0, 0, h0_ps[0])
                kv_drain(0, 1, h0_ps[1])
                kv_drain(1, 0, h0_ps[2])
                kv_drain(1, 1, h0_ps[3])
                rope_to(h0_ps[4][:], qT[0][:, 0:512], slice(0, 512))
                rope_to(h0_ps[5][:], qT[0][:, 512:1024], slice(512, 1024))

                # h1 token groups: K2 K3 V2 V3, same interleave
                h1_ps = [ps_proj.tile([128, 512], f32, tag="proj", name="h1ps0"),
                         ps_proj.tile([128, 512], f32, tag="proj", name="h1ps1"),
                         ps_sc.tile([128, 512], f32, tag="sc", name="h1ps2"),
                         ps_sc.tile([128, 512], f32, tag="sc", name="h1ps3")]
                for i in range(DCH):
                    group_mm(h1_ps[0], wkv_sb, 0, 2, i)
                    group_mm(h1_ps[1], wkv_sb, 0, 3, i)
                    group_mm(h1_ps[2], wkv_sb, 2048, 2, i)
                    group_mm(h1_ps[3], wkv_sb, 2048, 3, i)
                kv_drain(0, 2, h1_ps[0])
                kv_drain(0, 3, h1_ps[1])
                kv_drain(1, 2, h1_ps[2])
                kv_drain(1, 3, h1_ps[3])

                # V transposes: [128 dims, 128 keys] -> [128 keys, 128 dims]
                # (copies on ACT to keep DVE free for the attention loop)
                for kb in range(NKB):
                    tp = ps_proj.tile([128, 128], bf16, tag="proj")
                    nc.tensor.transpose(tp[:], vT[:, 128 * kb:128 * (kb + 1)],
                                        ident[:])
                    nc.scalar.copy(vones[0][:, 65 * kb:65 * kb + 64],
                                   tp[:, 0:64])
                    nc.scalar.copy(vones[1][:, 65 * kb:65 * kb + 64],
                                   tp[:, 64:128])

                # -------- phase 2: attention e0/e1 (q fills drain xw) ----
                att_etile(0, {0: [lambda: q_proj(0, 2)],
                              1: [lambda: q_proj(0, 3)],
                              4: [lambda: q_proj(1, 0)],
                              5: [lambda: q_proj(1, 1)],
                              6: [lambda: q_proj(1, 2)],
                              7: [lambda: q_proj(1, 3)]})
                att_etile(1, {0: [lambda: q_proj(2, 0)],
                              1: [lambda: q_proj(2, 1)],
                              2: [lambda: q_proj(2, 2)],
                              3: [lambda: q_proj(2, 3)],
                              4: [lambda: q_proj(3, 0)],
                              5: [lambda: q_proj(3, 1)],
                              6: [lambda: q_proj(3, 2)],
                              7: [lambda: q_proj(3, 3)]})
                # e0+e1 head-sharded outputs are complete: exchange now,
                # overlapping att(e2). With the warm-up A2A having absorbed
                # launch skew, rendezvous here is short.
                nc.gpsimd.collective_compute(
                    "AllToAll", mybir.AluOpType.bypass, replica_groups=rg,
                    ins=[ain01[:].opt()], outs=[aout01[:].opt()])

            # -------- phase 3: wo/aout staging + e2/e3 ------------------
            # e2 runs fill-free: its o-proj work would depend on the e0/e1
            # AllToAll, and the list scheduler interleaves fill matmuls
            # ahead of independent score matmuls in the PE FIFO — a
            # not-yet-finished exchange then head-blocks the whole e-tile.
            # All 16 o-proj feature chunks fill e3 instead (the exchange
            # is long done by then); pass2 overlaps the e3 AllToAll.
            with tc.tile_pool(name="wop", bufs=1) as wop, \
                 tc.tile_pool(name="yo", bufs=4) as yo:
                wo_sb = wop.tile([128, 8 * D], bf16, tag="wo")
                aout_ab = wop.tile([128, 8 * 512], bf16, tag="aout_ab")
                aout_c = wop.tile([128, 4 * 512], bf16, tag="aout_c")
                aout_d = wop.tile([128, 4 * 512], bf16, tag="aout_d")
                yhalf = wop.tile([128, 16 * 512], bf16, tag="yhalf")
                for jj in range(8):
                    nc.sync.dma_start(out=wo_sb[:, D * jj:D * (jj + 1)],
                                      in_=woT_ext[:, D * jj:D * (jj + 1)])

                # dst col 512*(4*(e%2)+gg) + 256*b2 = [b0 256 | b1 256]
                def aout_load(e):
                    at = (aout01, aout01, aout2, aout3)[e]
                    dst_t = (aout_ab, aout_ab, aout_c, aout_d)[e]
                    ecol = CH * e if e < 2 else 0
                    eblk = 4 * e if e < 2 else 0
                    for j in range(8):
                        b2, gg = j // 4, j % 4
                        dst = 512 * (eblk + gg) + 256 * b2
                        nc.sync.dma_start(
                            out=dst_t[:, dst:dst + 256],
                            in_=at[128 * j:128 * (j + 1), ecol:ecol + CH])

                aout_load(0)
                aout_load(1)

                att_etile(2)
                nc.gpsimd.collective_compute(
                    "AllToAll", mybir.AluOpType.bypass, replica_groups=rg,
                    ins=[ain2[:].opt()], outs=[aout2[:].opt()])

                def oproj_dt(dt):
                    """y.T chunk dt from features jj 0-7 (e0+e1) -> yhalf."""
                    ps = ps_proj.tile([128, 512], f32, tag="proj")
                    for jj in range(8):
                        nc.tensor.matmul(
                            ps[:],
                            wo_sb[:, D * jj + 128 * dt:D * jj + 128 * (dt + 1)],
                            aout_ab[:, 512 * jj:512 * (jj + 1)],
                            start=(jj == 0), stop=(jj == 7))
                    nc.scalar.copy(yhalf[:, 512 * dt:512 * (dt + 1)], ps[:])

                def op2(d0):
                    return lambda: (oproj_dt(d0), oproj_dt(d0 + 1))

                att_etile(3, {c: [op2(2 * c)] for c in range(8)})
                nc.gpsimd.collective_compute(
                    "AllToAll", mybir.AluOpType.bypass, replica_groups=rg,
                    ins=[ain3[:].opt()], outs=[aout3[:].opt()])
                aout_load(2)

                # wo jj 8-15 reuses wo_sb's slot once the e3 o-proj fills
                # have drained it
                wo_b = wop.tile([128, 8 * D], bf16, tag="wo")
                for jj in range(8):
                    nc.sync.dma_start(out=wo_b[:, D * jj:D * (jj + 1)],
                                      in_=woT_ext[:, D * (jj + 8):D * (jj + 9)])

                # pass2: e2's features accumulate while the e3 AllToAll runs
                for dt in range(DCH):
                    ps = ps_proj.tile([128, 512], f32, tag="proj")
                    for nn in range(4):
                        nc.tensor.matmul(
                            ps[:],
                            wo_b[:, D * nn + 128 * dt:D * nn + 128 * (dt + 1)],
                            aout_c[:, 512 * nn:512 * (nn + 1)],
                            start=(nn == 0), stop=(nn == 3))
                    nc.vector.tensor_add(yhalf[:, 512 * dt:512 * (dt + 1)],
                                         ps[:], yhalf[:, 512 * dt:512 * (dt + 1)])

                aout_load(3)
                # pass3: e3's features, final add + stream y.T out
                for dt in range(DCH):
                    ps = ps_proj.tile([128, 512], f32, tag="proj")
                    for nn in range(4):
                        nc.tensor.matmul(
                            ps[:],
                            wo_b[:, D * (nn + 4) + 128 * dt:D * (nn + 4) + 128 * (dt + 1)],
                            aout_d[:, 512 * nn:512 * (nn + 1)],
                            start=(nn == 0), stop=(nn == 3))
                    yv = yo.tile([128, 512], f32, tag="y")
                    nc.vector.tensor_add(yv[:], ps[:],
                                         yhalf[:, 512 * dt:512 * (dt + 1)])
                    nc.sync.dma_start(
                        out=yT_ext[:, 512 * dt:512 * (dt + 1)],
                        in_=yv[:])

    nc.compile()
    return nc


def _host_inputs(x, wq, wk, wv, wo):
    bf = ml_dtypes.bfloat16

    # xT per batch: [128, 16*2048]; img[p, 2048i + t] = x[b, t, 128i + p]
    xT = []
    for b in range(B):
        t = x[b].T.reshape(DCH, 128, L).transpose(1, 0, 2).reshape(128, DCH * L)
        xT.append(np.ascontiguousarray(t).astype(bf))

    # wq per group g: e-major image; col 2048e + 128i + r ; row = qrow(g,e,r)
    wqT = []
    for g in range(4):
        img = np.empty((128, 4 * D), np.float32)
        for e in range(4):
            rows = np.concatenate([
                np.arange(64) + 64 * (8 * g + e),
                np.arange(64) + 64 * (8 * g + 4 + e)])
            Wsel = wq[rows, :]                      # [128, 2048]
            blk = Wsel.T.reshape(DCH, 128, 128)     # [i, p, r]
            img[:, 2048 * e:2048 * (e + 1)] = \
                blk.transpose(1, 0, 2).reshape(128, 2048)
        wqT.append(np.ascontiguousarray(img).astype(bf))

    # wkv per group g: t-major (K then V); rows 128g..128g+128 of wk/wv
    wkvT = []
    for g in range(4):
        img = np.empty((128, 2 * D), np.float32)
        for t, W in enumerate((wk, wv)):
            Wsel = W[128 * g:128 * (g + 1), :]
            blk = Wsel.T.reshape(DCH, 128, 128)
            img[:, 2048 * t:2048 * (t + 1)] = \
                blk.transpose(1, 0, 2).reshape(128, 2048)
        wkvT.append(np.ascontiguousarray(img).astype(bf))

    # wo (shared): col 2048jj + eo, row p; f(jj=4e+r, p)
    forder = np.empty(2048, np.int64)
    for jj in range(16):
        e, r = jj // 4, jj % 4
        p = np.arange(128)
        head = np.where(p < 64, 8 * r + e, 8 * r + 4 + e)
        forder[128 * jj:128 * (jj + 1)] = 64 * head + (p % 64)
    Wsel = wo[:, forder]                            # [2048 eo, 2048 f]
    woT = np.ascontiguousarray(
        Wsel.T.reshape(16, 128, D).transpose(1, 0, 2).reshape(128, 16 * D)
    ).astype(bf)

    # rope tables [128, 2048]: row r -> dim d = r%64
    freqs = 1.0 / (THETA ** (np.arange(0, HEAD_DIM, 2, dtype=np.float32) / HEAD_DIM))
    pos = np.arange(L, dtype=np.float32)
    ph = np.outer(freqs, pos)                       # [32, L]
    cos64 = np.concatenate([np.cos(ph), np.cos(ph)], axis=0)   # [64, L]
    sin64 = np.concatenate([np.sin(ph), np.sin(ph)], axis=0)
    sgn = np.where(np.arange(64) < 32, -1.0, 1.0)[:, None].astype(np.float32)
    cosT = np.concatenate([cos64, cos64], axis=0).astype(bf)
    sinT = np.concatenate([sin64 * sgn, sin64 * sgn], axis=0).astype(bf)

    q_idx = np.arange(CH)
    k_idx = np.arange(128)
    m0 = np.where(k_idx[:, None] <= q_idx[None, :], 1.0, 0.0)
    m1 = np.where(k_idx[:, None] + 128 <= q_idx[None, :], 1.0, 0.0)
    mask = np.concatenate([m0, m1], axis=1).astype(bf)

    in_maps = []
    for c in range(N_CORES):
        b, g = c // 4, c % 4
        in_maps.append({
            "xT": xT[b], "wqT": wqT[g], "wkvT": wkvT[g], "woT": woT,
            "cosT": cosT, "sinT": sinT, "maskT": mask,
        })
    return in_maps


def kernel(x, wq, wk, wv, wo):
    from concourse.bass_utils import run_bass_kernel_spmd

    if "nc" not in _BUILT:
        _BUILT["nc"] = _build_nc()
    nc = _BUILT["nc"]

    in_maps = _host_inputs(np.asarray(x), np.asarray(wq), np.asarray(wk),
                           np.asarray(wv), np.asarray(wo))
    trace = bool(os.environ.get("BASS_KERNEL_TRACE"))
    res = run_bass_kernel_spmd(nc, in_maps, core_ids=list(range(N_CORES)),
                               trace=trace)
    kernel.last_exec_time_ns = res.exec_time_ns
    kernel.last_results = res

    y = np.empty((B, L, D), dtype=np.float32)
    for c in range(N_CORES):
        arr = res.results[c]["yT"].reshape(128, DCH, 2, CH)
        for b2 in range(2):
            y[b2, CH * c:CH * (c + 1), :] = \
                arr[:, :, b2, :].transpose(2, 1, 0).reshape(CH, D)
    return y
